# revision 1
# baseline (speedup 1.0000x reference)
"""Trainium2 Bass kernel for block-causal sparse attention (MLA-style KV).

Sharding: tensor-parallel over heads. 16 heads / 8 cores = 2 heads per core,
one KV head per core-pair. Each core computes q/k/v projections from the full
(transposed) x, RoPE, sparse attention for its 2 heads, and a partial output
projection; the host sums the 8 partial outputs.

Sparsity structure (T=4096, BLOCK=128, WINDOW=512, GLOBAL_EVERY=64):
for query block b, visible keys are blocks b-4..b (block b-4 masked by a fixed
triangular+global pattern) plus "global" columns j%64==0 with j < 128*(b-4).

All matmuls run as float32r (TF32-like, ~1.5e-4 rel err, full PE speed).
Scores are computed transposed ([k, q] layout) so probabilities feed the PV
and output-projection matmuls with no transposes. Softmax denominators are
computed with ones-matmul partition reductions accumulated in PSUM; the
reciprocal is broadcast across partitions on GPSIMD.
"""

import numpy as np

N_CORES = 8
T = 4096
C = 2048
L = 512
H = 16
KVH = 4
HD = 128
BLOCK = 128
WINDOW = 512
GLOBAL_EVERY = 64
ROPE_THETA = 10000.0

QTW = 512            # query tile width (4 blocks)
NQT = T // QTW       # 8
NKT = C // 128       # 16 contraction tiles for projections
NNT = T // 512       # 8 t-tiles for projections
NG = T // GLOBAL_EVERY  # 64 global columns

_CACHE = {}


def _build_module():
    import concourse.bacc as bacc
    import concourse.mybir as mybir
    import concourse.tile as tile
    from contextlib import ExitStack

    F32 = mybir.dt.float32
    F32R = mybir.dt.float32r
    EXP = mybir.ActivationFunctionType.Exp

    nc = bacc.Bacc("TRN2", target_bir_lowering=False, debug=False,
                   num_devices=N_CORES)

    xt = nc.dram_tensor("xt", [C, T], F32R, kind="ExternalInput")
    wq = nc.dram_tensor("wq", [C, 2 * HD], F32R, kind="ExternalInput")
    wk = nc.dram_tensor("wk", [C, HD], F32R, kind="ExternalInput")
    wv = nc.dram_tensor("wv", [C, HD], F32R, kind="ExternalInput")
    wo = nc.dram_tensor("wo", [2 * HD, C], F32R, kind="ExternalInput")
    cosd = nc.dram_tensor("cosd", [HD, T], F32, kind="ExternalInput")
    sind = nc.dram_tensor("sind", [HD, T], F32, kind="ExternalInput")  # sign-folded
    maskt = nc.dram_tensor("maskt", [128, 128], F32, kind="ExternalInput")
    maskg = nc.dram_tensor("maskg", [NG, T], mybir.dt.bfloat16, kind="ExternalInput")
    onesd = nc.dram_tensor("onesd", [128, 1], F32R, kind="ExternalInput")
    identd = nc.dram_tensor("identd", [128, 128], F32R, kind="ExternalInput")
    out = nc.dram_tensor("out", [T, C], F32, kind="ExternalOutput")

    scale = 1.0 / np.sqrt(HD)

    with tile.TileContext(nc) as tc, ExitStack() as ctx:
        res = ctx.enter_context(tc.tile_pool(name="res", bufs=1))
        kT = res.tile([128, T], F32R, tag="kT")
        vN = res.tile([128, T], F32R, tag="vN")
        kG = res.tile([128, NG], F32R, tag="kG")
        vG = res.tile([64, 128], F32R, tag="vG")
        vGT = res.tile([128, NG], F32R, tag="vGT")
        mT = res.tile([128, 128], F32, tag="mT")
        mG = res.tile([NG, T], mybir.dt.bfloat16, tag="mG")
        ones = res.tile([128, 1], F32R, tag="ones")
        ident = res.tile([128, 128], F32R, tag="ident")
        wo_sb = res.tile([128, 2 * C], F32R, tag="wo_sb")
        wq_sb = res.tile([128, NKT * 256], F32R, tag="wq_sb")
        wk_sb = res.tile([128, NKT * 128], F32R, tag="wk_sb")
        wv_sb = res.tile([128, NKT * 128], F32R, tag="wv_sb")

        xpool = ctx.enter_context(tc.tile_pool(name="xpool", bufs=22))
        qlp = ctx.enter_context(tc.tile_pool(name="qlp", bufs=2))
        csp = ctx.enter_context(tc.tile_pool(name="csp", bufs=2))
        vtp = ctx.enter_context(tc.tile_pool(name="vtp", bufs=2))
        swp = ctx.enter_context(tc.tile_pool(name="swp", bufs=2))
        tmpp = ctx.enter_context(tc.tile_pool(name="tmpp", bufs=2))
        ppool = ctx.enter_context(tc.tile_pool(name="ppool", bufs=3))
        ynp = ctx.enter_context(tc.tile_pool(name="ynp", bufs=2))
        recp = ctx.enter_context(tc.tile_pool(name="recp", bufs=2))
        rbcp = ctx.enter_context(tc.tile_pool(name="rbcp", bufs=2))
        obp = ctx.enter_context(tc.tile_pool(name="obp", bufs=3))

        pjps = ctx.enter_context(tc.tile_pool(name="pjps", bufs=2, space="PSUM"))
        spool = ctx.enter_context(tc.tile_pool(name="spool", bufs=2, space="PSUM"))
        ypool = ctx.enter_context(tc.tile_pool(name="ypool", bufs=1, space="PSUM"))
        dpool = ctx.enter_context(tc.tile_pool(name="dpool", bufs=1, space="PSUM"))
        opool = ctx.enter_context(tc.tile_pool(name="opool", bufs=2, space="PSUM"))

        def emit_wo(ynorm, qs0):
            for qs in range(4):
                rows = slice(qs0 + qs * 128, qs0 + (qs + 1) * 128)
                for n in range(4):
                    o_ps = opool.tile([128, 512], F32, tag="o", name="o_ps")
                    nc.tensor.matmul(o_ps[:], ynorm[0][:, qs * 128:(qs + 1) * 128],
                                     wo_sb[:, n * 512:n * 512 + 512],
                                     start=True, stop=False)
                    nc.tensor.matmul(o_ps[:], ynorm[1][:, qs * 128:(qs + 1) * 128],
                                     wo_sb[:, C + n * 512:C + n * 512 + 512],
                                     start=False, stop=True)
                    ob = obp.tile([128, 512], F32, tag="ob", name="ob")
                    if (qs * 4 + n) % 4 != 3:
                        nc.scalar.copy(ob[:], o_ps[:])
                    else:
                        nc.vector.tensor_copy(ob[:], o_ps[:])
                    nc.sync.dma_start(out[rows, n * 512:(n + 1) * 512], ob[:])

        pending_wo = None
        for it in range(NQT):
            nt = it
            b0 = 4 * it
            ts = slice(nt * 512, (nt + 1) * 512)
            qs0 = it * QTW

            # ---- projections for t-tile `nt` (q0, q1, k, v sequentially
            # through 2 PSUM slots; all 16 x-tiles stay resident in SBUF) ----
            xts = []
            cos_t = csp.tile([128, 512], F32, tag="cos")
            sin_t = csp.tile([128, 512], F32, tag="sin")
            nc.sync.dma_start(cos_t[:], cosd[:, ts])
            nc.sync.dma_start(sin_t[:], sind[:, ts])
            for kt in range(NKT):
                if it == 0:
                    nc.sync.dma_start(wq_sb[:, kt * 256:(kt + 1) * 256],
                                      wq[kt * 128:(kt + 1) * 128, :])
                    if kt == 0:
                        nc.gpsimd.dma_start(ident[:], identd[:])
                        nc.gpsimd.dma_start(mT[:], maskt[:])
                        nc.gpsimd.dma_start(ones[:], onesd[:])
                xtile = xpool.tile([128, 512], F32R, tag="xtile")
                nc.sync.dma_start(xtile[:], xt[kt * 128:(kt + 1) * 128, ts])
                xts.append(xtile)
            if it == 0:
                # k/v weights are first needed two PSUM passes later; keep
                # them off the q-projection critical DMA path
                for kt in range(NKT):
                    nc.sync.dma_start(wk_sb[:, kt * 128:(kt + 1) * 128],
                                      wk[kt * 128:(kt + 1) * 128, :])
                    nc.sync.dma_start(wv_sb[:, kt * 128:(kt + 1) * 128],
                                      wv[kt * 128:(kt + 1) * 128, :])

            qloc = [qlp.tile([128, 512], F32R, tag=f"ql{h}", name=f"ql{h}")
                    for h in range(2)]
            wslices = [
                lambda kt: wq_sb[:, kt * 256:kt * 256 + 128],
                lambda kt: wq_sb[:, kt * 256 + 128:kt * 256 + 256],
                lambda kt: wk_sb[:, kt * 128:(kt + 1) * 128],
                lambda kt: wv_sb[:, kt * 128:(kt + 1) * 128],
            ]
            vT_t = vtp.tile([128, 512], F32R, tag="vT")
            for i in range(4):
                pj = pjps.tile([128, 512], F32, tag="pj")
                for kt in range(NKT):
                    nc.tensor.matmul(pj[:], wslices[i](kt), xts[kt][:],
                                     start=(kt == 0), stop=(kt == NKT - 1))
                if i < 3:
                    # RoPE: dest = pj*cos + swap(pj)*sinS
                    dest = qloc[i][:] if i < 2 else kT[:, ts]
                    qsb = swp.tile([128, 512], F32, tag="qsb")
                    nc.scalar.copy(qsb[:], pj[:])
                    sw = swp.tile([128, 512], F32, tag="sw")
                    nc.gpsimd.dma_start(sw[0:64, :], qsb[64:128, :])
                    nc.gpsimd.dma_start(sw[64:128, :], qsb[0:64, :])
                    ta = tmpp.tile([128, 512], F32, tag="ta")
                    nc.vector.tensor_mul(ta[:], pj[:], cos_t[:])
                    tb = tmpp.tile([128, 512], F32, tag="tb")
                    nc.vector.tensor_mul(tb[:], sw[:], sin_t[:])
                    nc.vector.tensor_add(dest, ta[:], tb[:])
                else:
                    nc.vector.tensor_copy(vT_t[:], pj[:])

            if it == 0:
                nc.gpsimd.dma_start(mG[:], maskg[:])
                for i in range(2):
                    nc.sync.dma_start(wo_sb[:, i * C:(i + 1) * C],
                                      wo[i * 128:(i + 1) * 128, :])

            # ---- v transpose for this t-tile + incremental global K/V ----
            for j in range(4):
                blk = nt * 4 + j
                tp = spool.tile([128, 512], F32R, tag="s", name="tp")
                nc.tensor.transpose(tp[:, :128], vT_t[:, j * 128:(j + 1) * 128],
                                    ident[:])
                nc.scalar.copy(vN[:, blk * 128:(blk + 1) * 128], tp[:, :128])
            gsl = slice(nt * 8, (nt + 1) * 8)
            nc.vector.tensor_copy(kG[:, gsl], kT[:, ts][:, 0:512:GLOBAL_EVERY])
            nc.vector.tensor_copy(vGT[:, gsl], vT_t[:][:, 0:512:GLOBAL_EVERY])
            gw2 = 8 * (nt + 1)
            tpg = spool.tile([128, 512], F32R, tag="s", name="tpg")
            nc.tensor.transpose(tpg[:gw2, :128], vGT[:, :gw2], ident[:])
            nc.vector.tensor_copy(vG[:gw2, :], tpg[:gw2, :128])

            # ---- attention for query tile `it` (4 blocks b0..b0+3) ----
            gw = min(NG, 8 * it)   # written prefix of kG/vG; 0 for it=0
            ynorm = []
            for h in range(2):
                if h == 1 and pending_wo is not None:
                    emit_wo(*pending_wo)
                    pending_wo = None
                items = [(b0, 0, 512, None)]
                if it == 0:
                    for j in range(3):
                        items.append((j + 1, (j + 1) * 128, (3 - j) * 128, None))
                    use_glob = False
                else:
                    for j in range(4):
                        items.append((b0 - 4 + j, 0, (j + 1) * 128, j))
                    for j in range(3):
                        items.append((b0 + 1 + j, (j + 1) * 128, (3 - j) * 128, None))
                    use_glob = gw > 0

                y_ps = ypool.tile([128, QTW], F32, tag="y")
                d_ps = dpool.tile([1, QTW], F32, tag="d")
                n_items = len(items) + (1 if use_glob else 0)
                s_tiles = [None] * n_items

                def emit_qk(ii):
                    s = spool.tile([128, QTW], F32, tag="s")
                    if ii < len(items):
                        kb, qoff, w, _ = items[ii]
                        nc.tensor.matmul(
                            s[:, :w], kT[:, kb * 128:(kb + 1) * 128],
                            qloc[h][:, qoff:qoff + w],
                            start=True, stop=True)
                    else:
                        nc.tensor.matmul(s[:gw, :], kG[:, :gw], qloc[h][:],
                                         start=True, stop=True)
                    s_tiles[ii] = s

                def emit_rest(ii):
                    first = ii == 0
                    last = ii == n_items - 1
                    s = s_tiles[ii]
                    p = ppool.tile([128, QTW], F32R, tag="p")
                    if ii < len(items):
                        kb, qoff, w, tri = items[ii]
                        nc.scalar.activation(p[:, :w], s[:, :w], EXP, scale=scale)
                        if tri is not None:
                            nc.vector.tensor_mul(p[:, tri * 128:(tri + 1) * 128],
                                                 p[:, tri * 128:(tri + 1) * 128],
                                                 mT[:])
                        nc.tensor.matmul(y_ps[:, qoff:qoff + w],
                                         vN[:, kb * 128:(kb + 1) * 128], p[:, :w],
                                         start=first, stop=last)
                        nc.tensor.matmul(d_ps[:, qoff:qoff + w], ones[:, :],
                                         p[:, :w], start=first, stop=last)
                    else:
                        nc.scalar.activation(p[:gw, :], s[:gw, :], EXP, scale=scale)
                        nc.vector.tensor_mul(p[:gw, :], p[:gw, :],
                                             mG[:gw, qs0:qs0 + QTW])
                        nc.tensor.matmul(y_ps[:, :], vG[:gw, :], p[:gw, :],
                                         start=first, stop=last)
                        nc.tensor.matmul(d_ps[:, :], ones[:gw, :], p[:gw, :],
                                         start=first, stop=last)

                emit_qk(0)
                for ii in range(n_items):
                    if ii + 1 < n_items:
                        emit_qk(ii + 1)
                    emit_rest(ii)

                rec = recp.tile([1, QTW], F32, tag="rec")
                nc.vector.reciprocal(rec[:], d_ps[:])
                rbc = rbcp.tile([128, QTW], F32, tag="rbc")
                nc.gpsimd.partition_broadcast(rbc[:], rec[:])
                yn = ynp.tile([128, QTW], F32R, tag=f"yn{h}", name=f"yn{h}")
                nc.vector.tensor_mul(yn[:], y_ps[:], rbc[:])
                ynorm.append(yn)

            # ---- output projection: deferred to overlap with the next
            # iteration's projection matmuls (hides the normalize latency) ----
            pending_wo = (ynorm, qs0)

        emit_wo(*pending_wo)

    nc.compile()
    return nc


def _host_inputs(x, w_q, w_kv_down, w_k_up, w_v_up, w_o):
    """Build the per-core input maps (host-side shard + precompute)."""
    x = np.asarray(x)
    w_q = np.asarray(w_q)
    w_kv_down = np.asarray(w_kv_down)
    w_k_up = np.asarray(w_k_up)
    w_v_up = np.asarray(w_v_up)
    w_o = np.asarray(w_o)
    x2 = np.ascontiguousarray(x.reshape(T, C).astype(np.float32))
    xt = np.ascontiguousarray(x2.T)

    # RoPE tables, [hd, t] layout, sign folded into sin for the swapped term
    freqs = 1.0 / (ROPE_THETA ** (np.arange(0, HD, 2, dtype=np.float64) / HD))
    emb = np.arange(T, dtype=np.float64)[:, None] * freqs[None, :]   # [T, 64]
    cos = np.concatenate([np.cos(emb), np.cos(emb)], axis=-1)        # [T, 128]
    sin = np.concatenate([np.sin(emb), np.sin(emb)], axis=-1)
    cosT = np.ascontiguousarray(cos.T.astype(np.float32))            # [128, T]
    sinS = sin.T.copy()
    sinS[:64, :] *= -1.0
    sinS = np.ascontiguousarray(sinS.astype(np.float32))

    # fixed triangular+global mask for the b-4 key block, [k_off, q_off]
    oi = np.arange(128)
    mT = ((oi[None, :] <= oi[:, None]) | (oi[:, None] % 64 == 0)).astype(np.float32)

    # global-column mask [g, q]: visible iff 64 g < 128 (q//128 - 4)
    g = np.arange(NG)
    qb = np.arange(T) // BLOCK
    import ml_dtypes
    mG = (64 * g[:, None] < 128 * (qb[None, :] - 4)).astype(ml_dtypes.bfloat16)

    onesv = np.ones((128, 1), np.float32)
    ident = np.eye(128, dtype=np.float32)

    wk_f = (w_kv_down.astype(np.float32) @ w_k_up.astype(np.float32))  # [C, KVH*HD]
    wv_f = (w_kv_down.astype(np.float32) @ w_v_up.astype(np.float32))

    in_maps = []
    for c in range(N_CORES):
        h0 = 2 * c
        kv = h0 // (H // KVH)
        wq_c = np.ascontiguousarray(
            w_q[:, h0 * HD:(h0 + 2) * HD].astype(np.float32))
        wk_c = np.ascontiguousarray(
            wk_f[:, kv * HD:(kv + 1) * HD].astype(np.float32))
        wv_c = np.ascontiguousarray(
            wv_f[:, kv * HD:(kv + 1) * HD].astype(np.float32))
        wo_c = np.ascontiguousarray(
            w_o[h0 * HD:(h0 + 2) * HD, :].astype(np.float32))
        in_maps.append({
            "xt": xt, "wq": wq_c, "wk": wk_c, "wv": wv_c, "wo": wo_c,
            "cosd": cosT, "sind": sinS, "maskt": mT, "maskg": mG,
            "onesd": onesv, "identd": ident,
        })
    return in_maps


def _get_module():
    if "nc" not in _CACHE:
        _CACHE["nc"] = _build_module()
    return _CACHE["nc"]


def kernel(x, w_q, w_kv_down, w_k_up, w_v_up, w_o):
    from concourse.bass_utils import run_bass_kernel_spmd

    nc = _get_module()
    in_maps = _host_inputs(x, w_q, w_kv_down, w_k_up, w_v_up, w_o)
    res = run_bass_kernel_spmd(nc, in_maps, list(range(N_CORES)))
    acc = np.zeros((T, C), np.float32)
    for c in range(N_CORES):
        acc += res.results[c]["out"]
    return acc.reshape(1, T, C)



# revision 2
# speedup vs baseline: 1.0074x; 1.0074x over previous
"""Trainium2 Bass kernel for block-causal sparse attention (MLA-style KV).

Sharding: tensor-parallel over heads. 16 heads / 8 cores = 2 heads per core,
one KV head per core-pair. Each core computes q/k/v projections from the full
(transposed) x, RoPE, sparse attention for its 2 heads, and a partial output
projection; the host sums the 8 partial outputs.

Sparsity structure (T=4096, BLOCK=128, WINDOW=512, GLOBAL_EVERY=64):
for query block b, visible keys are blocks b-4..b (block b-4 masked by a fixed
triangular+global pattern) plus "global" columns j%64==0 with j < 128*(b-4).

All matmul operands are bf16 (fp32 PSUM accumulation); bf16 keeps the PE at
1 cycle/row for every tile width and halves DMA bytes and DVE element time.
Scores are computed transposed ([k, q] layout) so probabilities feed the PV
and output-projection matmuls with no transposes. Softmax denominators are
accumulated with ones-matrix matmuls directly in broadcast form ([128, q] in
PSUM), so the reciprocal feeds the normalize multiply without a partition
broadcast.
"""

import numpy as np

N_CORES = 8
T = 4096
C = 2048
L = 512
H = 16
KVH = 4
HD = 128
BLOCK = 128
WINDOW = 512
GLOBAL_EVERY = 64
ROPE_THETA = 10000.0

QTW = 512            # query tile width (4 blocks)
NQT = T // QTW       # 8
NKT = C // 128       # 16 contraction tiles for projections
NNT = T // 512       # 8 t-tiles for projections
NG = T // GLOBAL_EVERY  # 64 global columns

_CACHE = {}


def _build_module():
    import concourse.bacc as bacc
    import concourse.mybir as mybir
    import concourse.tile as tile
    from contextlib import ExitStack

    F32 = mybir.dt.float32
    BF16 = mybir.dt.bfloat16
    EXP = mybir.ActivationFunctionType.Exp

    nc = bacc.Bacc("TRN2", target_bir_lowering=False, debug=False,
                   num_devices=N_CORES)

    xt = nc.dram_tensor("xt", [C, T], BF16, kind="ExternalInput")
    wq = nc.dram_tensor("wq", [C, 2 * HD], BF16, kind="ExternalInput")
    wk = nc.dram_tensor("wk", [C, HD], BF16, kind="ExternalInput")
    wv = nc.dram_tensor("wv", [C, HD], BF16, kind="ExternalInput")
    wo = nc.dram_tensor("wo", [2 * HD, C], BF16, kind="ExternalInput")
    cosd = nc.dram_tensor("cosd", [HD, T], BF16, kind="ExternalInput")
    sind = nc.dram_tensor("sind", [HD, T], BF16, kind="ExternalInput")  # sign-folded
    maskt = nc.dram_tensor("maskt", [128, 128], BF16, kind="ExternalInput")
    maskg = nc.dram_tensor("maskg", [NG, T], BF16, kind="ExternalInput")
    onesd = nc.dram_tensor("onesd", [128, 128], BF16, kind="ExternalInput")
    identd = nc.dram_tensor("identd", [128, 128], BF16, kind="ExternalInput")
    out = nc.dram_tensor("out", [T, C], BF16, kind="ExternalOutput")

    scale = 1.0 / np.sqrt(HD)

    with tile.TileContext(nc) as tc, ExitStack() as ctx:
        res = ctx.enter_context(tc.tile_pool(name="res", bufs=1))
        kT = res.tile([128, T], BF16, tag="kT")
        vN = res.tile([128, T], BF16, tag="vN")
        kG = res.tile([128, NG], BF16, tag="kG")
        vG = res.tile([64, 128], BF16, tag="vG")
        vGT = res.tile([128, NG], BF16, tag="vGT")
        mT = res.tile([128, 128], BF16, tag="mT")
        mG = res.tile([NG, T], BF16, tag="mG")
        ones = res.tile([128, 128], BF16, tag="ones")
        ident = res.tile([128, 128], BF16, tag="ident")
        wo_sb = res.tile([128, 2 * C], BF16, tag="wo_sb")
        wq_sb = res.tile([128, NKT * 256], BF16, tag="wq_sb")
        wk_sb = res.tile([128, NKT * 128], BF16, tag="wk_sb")
        wv_sb = res.tile([128, NKT * 128], BF16, tag="wv_sb")

        xpool = ctx.enter_context(tc.tile_pool(name="xpool", bufs=36))
        qlp = ctx.enter_context(tc.tile_pool(name="qlp", bufs=2))
        csp = ctx.enter_context(tc.tile_pool(name="csp", bufs=2))
        vtp = ctx.enter_context(tc.tile_pool(name="vtp", bufs=2))
        swp = ctx.enter_context(tc.tile_pool(name="swp", bufs=2))
        tmpp = ctx.enter_context(tc.tile_pool(name="tmpp", bufs=2))
        ppool = ctx.enter_context(tc.tile_pool(name="ppool", bufs=3))
        ynp = ctx.enter_context(tc.tile_pool(name="ynp", bufs=2))
        recp = ctx.enter_context(tc.tile_pool(name="recp", bufs=2))
        obp = ctx.enter_context(tc.tile_pool(name="obp", bufs=3))

        pjps = ctx.enter_context(tc.tile_pool(name="pjps", bufs=2, space="PSUM"))
        spool = ctx.enter_context(tc.tile_pool(name="spool", bufs=2, space="PSUM"))
        ypool = ctx.enter_context(tc.tile_pool(name="ypool", bufs=1, space="PSUM"))
        dpool = ctx.enter_context(tc.tile_pool(name="dpool", bufs=1, space="PSUM"))
        opool = ctx.enter_context(tc.tile_pool(name="opool", bufs=2, space="PSUM"))

        def emit_wo(ynorm, qs0):
            for qs in range(4):
                rows = slice(qs0 + qs * 128, qs0 + (qs + 1) * 128)
                for n in range(4):
                    o_ps = opool.tile([128, 512], F32, tag="o", name="o_ps")
                    nc.tensor.matmul(o_ps[:], ynorm[0][:, qs * 128:(qs + 1) * 128],
                                     wo_sb[:, n * 512:n * 512 + 512],
                                     start=True, stop=False)
                    nc.tensor.matmul(o_ps[:], ynorm[1][:, qs * 128:(qs + 1) * 128],
                                     wo_sb[:, C + n * 512:C + n * 512 + 512],
                                     start=False, stop=True)
                    ob = obp.tile([128, 512], BF16, tag="ob", name="ob")
                    if (qs * 4 + n) % 4 != 3:
                        nc.scalar.copy(ob[:], o_ps[:])
                    else:
                        nc.vector.tensor_copy(ob[:], o_ps[:])
                    nc.sync.dma_start(out[rows, n * 512:(n + 1) * 512], ob[:])

        pending_wo = None
        for it in range(NQT):
            nt = it
            b0 = 4 * it
            ts = slice(nt * 512, (nt + 1) * 512)
            qs0 = it * QTW

            # ---- projections for t-tile `nt` (q0, q1, k, v sequentially
            # through 2 PSUM slots; all 16 x-tiles stay resident in SBUF) ----
            xts = []
            cos_t = csp.tile([128, 512], BF16, tag="cos")
            sin_t = csp.tile([128, 512], BF16, tag="sin")
            nc.sync.dma_start(cos_t[:], cosd[:, ts])
            nc.sync.dma_start(sin_t[:], sind[:, ts])
            for kt in range(NKT):
                if it == 0:
                    nc.sync.dma_start(wq_sb[:, kt * 256:(kt + 1) * 256],
                                      wq[kt * 128:(kt + 1) * 128, :])
                    if kt == 0:
                        nc.gpsimd.dma_start(ident[:], identd[:])
                        nc.gpsimd.dma_start(mT[:], maskt[:])
                        nc.gpsimd.dma_start(ones[:], onesd[:])
                xtile = xpool.tile([128, 512], BF16, tag="xtile")
                nc.sync.dma_start(xtile[:], xt[kt * 128:(kt + 1) * 128, ts])
                xts.append(xtile)
            if it == 0:
                # k/v weights are first needed two PSUM passes later; keep
                # them off the q-projection critical DMA path
                for kt in range(NKT):
                    nc.sync.dma_start(wk_sb[:, kt * 128:(kt + 1) * 128],
                                      wk[kt * 128:(kt + 1) * 128, :])
                    nc.sync.dma_start(wv_sb[:, kt * 128:(kt + 1) * 128],
                                      wv[kt * 128:(kt + 1) * 128, :])

            qloc = [qlp.tile([128, 512], BF16, tag=f"ql{h}", name=f"ql{h}")
                    for h in range(2)]
            wslices = [
                lambda kt: wq_sb[:, kt * 256:kt * 256 + 128],
                lambda kt: wq_sb[:, kt * 256 + 128:kt * 256 + 256],
                lambda kt: wk_sb[:, kt * 128:(kt + 1) * 128],
                lambda kt: wv_sb[:, kt * 128:(kt + 1) * 128],
            ]
            vT_t = vtp.tile([128, 512], BF16, tag="vT")
            for i in range(4):
                pj = pjps.tile([128, 512], F32, tag="pj")
                for kt in range(NKT):
                    nc.tensor.matmul(pj[:], wslices[i](kt), xts[kt][:],
                                     start=(kt == 0), stop=(kt == NKT - 1))
                if i < 3:
                    # RoPE: dest = qsb*cos + swap(qsb)*sinS
                    dest = qloc[i][:] if i < 2 else kT[:, ts]
                    qsb = swp.tile([128, 512], BF16, tag="qsb")
                    nc.scalar.copy(qsb[:], pj[:])
                    sw = swp.tile([128, 512], BF16, tag="sw")
                    nc.gpsimd.dma_start(sw[0:64, :], qsb[64:128, :])
                    nc.gpsimd.dma_start(sw[64:128, :], qsb[0:64, :])
                    ta = tmpp.tile([128, 512], BF16, tag="ta")
                    nc.vector.tensor_mul(ta[:], qsb[:], cos_t[:])
                    tb = tmpp.tile([128, 512], BF16, tag="tb")
                    nc.vector.tensor_mul(tb[:], sw[:], sin_t[:])
                    nc.vector.tensor_add(dest, ta[:], tb[:])
                else:
                    nc.vector.tensor_copy(vT_t[:], pj[:])

            if it == 0:
                nc.gpsimd.dma_start(mG[:], maskg[:])
                for i in range(2):
                    nc.sync.dma_start(wo_sb[:, i * C:(i + 1) * C],
                                      wo[i * 128:(i + 1) * 128, :])

            # ---- v transpose for this t-tile + incremental global K/V ----
            for j in range(4):
                blk = nt * 4 + j
                tp = spool.tile([128, 512], BF16, tag="s", name="tp")
                nc.tensor.transpose(tp[:, :128], vT_t[:, j * 128:(j + 1) * 128],
                                    ident[:])
                nc.vector.tensor_copy(vN[:, blk * 128:(blk + 1) * 128], tp[:, :128])
            gsl = slice(nt * 8, (nt + 1) * 8)
            nc.vector.tensor_copy(kG[:, gsl], kT[:, ts][:, 0:512:GLOBAL_EVERY])
            nc.vector.tensor_copy(vGT[:, gsl], vT_t[:][:, 0:512:GLOBAL_EVERY])
            gw2 = 8 * (nt + 1)
            tpg = spool.tile([128, 512], BF16, tag="s", name="tpg")
            nc.tensor.transpose(tpg[:gw2, :128], vGT[:, :gw2], ident[:])
            nc.vector.tensor_copy(vG[:gw2, :], tpg[:gw2, :128])

            # ---- attention for query tile `it` (4 blocks b0..b0+3) ----
            gw = min(NG, 8 * it)   # written prefix of kG/vG; 0 for it=0
            ynorm = []
            for h in range(2):
                if h == 1 and pending_wo is not None:
                    emit_wo(*pending_wo)
                    pending_wo = None
                items = [(b0, 0, 512, None)]
                if it == 0:
                    for j in range(3):
                        items.append((j + 1, (j + 1) * 128, (3 - j) * 128, None))
                    use_glob = False
                else:
                    for j in range(4):
                        items.append((b0 - 4 + j, 0, (j + 1) * 128, j))
                    for j in range(3):
                        items.append((b0 + 1 + j, (j + 1) * 128, (3 - j) * 128, None))
                    use_glob = gw > 0

                y_ps = ypool.tile([128, QTW], F32, tag="y")
                d_ps = dpool.tile([128, QTW], F32, tag="d")
                n_items = len(items) + (1 if use_glob else 0)
                s_tiles = [None] * n_items

                def emit_qk(ii):
                    s = spool.tile([128, QTW], F32, tag="s")
                    if ii < len(items):
                        kb, qoff, w, _ = items[ii]
                        nc.tensor.matmul(
                            s[:, :w], kT[:, kb * 128:(kb + 1) * 128],
                            qloc[h][:, qoff:qoff + w],
                            start=True, stop=True)
                    else:
                        nc.tensor.matmul(s[:gw, :], kG[:, :gw], qloc[h][:],
                                         start=True, stop=True)
                    s_tiles[ii] = s

                def emit_rest(ii):
                    first = ii == 0
                    last = ii == n_items - 1
                    s = s_tiles[ii]
                    p = ppool.tile([128, QTW], BF16, tag="p")
                    if ii < len(items):
                        kb, qoff, w, tri = items[ii]
                        nc.scalar.activation(p[:, :w], s[:, :w], EXP, scale=scale)
                        if tri is not None:
                            nc.vector.tensor_mul(p[:, tri * 128:(tri + 1) * 128],
                                                 p[:, tri * 128:(tri + 1) * 128],
                                                 mT[:])
                        nc.tensor.matmul(y_ps[:, qoff:qoff + w],
                                         vN[:, kb * 128:(kb + 1) * 128], p[:, :w],
                                         start=first, stop=last)
                        nc.tensor.matmul(d_ps[:, qoff:qoff + w], ones[:, :],
                                         p[:, :w], start=first, stop=last)
                    else:
                        nc.scalar.activation(p[:gw, :], s[:gw, :], EXP, scale=scale)
                        nc.vector.tensor_mul(p[:gw, :], p[:gw, :],
                                             mG[:gw, qs0:qs0 + QTW])
                        nc.tensor.matmul(y_ps[:, :], vG[:gw, :], p[:gw, :],
                                         start=first, stop=last)
                        nc.tensor.matmul(d_ps[:, :], ones[:gw, :], p[:gw, :],
                                         start=first, stop=last)

                emit_qk(0)
                for ii in range(n_items):
                    if ii + 1 < n_items:
                        emit_qk(ii + 1)
                    emit_rest(ii)

                # d_ps holds the denominator replicated across partitions, so
                # the reciprocal is already in broadcast form for the multiply
                rbc = recp.tile([128, QTW], F32, tag="rbc")
                nc.vector.reciprocal(rbc[:], d_ps[:])
                yn = ynp.tile([128, QTW], BF16, tag=f"yn{h}", name=f"yn{h}")
                nc.vector.tensor_mul(yn[:], y_ps[:], rbc[:])
                ynorm.append(yn)

            # ---- output projection: deferred to overlap with the next
            # iteration's projection matmuls (hides the normalize latency) ----
            pending_wo = (ynorm, qs0)

        emit_wo(*pending_wo)

    nc.compile()
    return nc


def _host_inputs(x, w_q, w_kv_down, w_k_up, w_v_up, w_o):
    """Build the per-core input maps (host-side shard + precompute)."""
    import ml_dtypes
    BF = ml_dtypes.bfloat16
    x = np.asarray(x)
    w_q = np.asarray(w_q)
    w_kv_down = np.asarray(w_kv_down)
    w_k_up = np.asarray(w_k_up)
    w_v_up = np.asarray(w_v_up)
    w_o = np.asarray(w_o)
    x2 = np.ascontiguousarray(x.reshape(T, C).astype(np.float32))
    xt = np.ascontiguousarray(x2.T.astype(BF))

    # RoPE tables, [hd, t] layout, sign folded into sin for the swapped term
    freqs = 1.0 / (ROPE_THETA ** (np.arange(0, HD, 2, dtype=np.float64) / HD))
    emb = np.arange(T, dtype=np.float64)[:, None] * freqs[None, :]   # [T, 64]
    cos = np.concatenate([np.cos(emb), np.cos(emb)], axis=-1)        # [T, 128]
    sin = np.concatenate([np.sin(emb), np.sin(emb)], axis=-1)
    cosT = np.ascontiguousarray(cos.T.astype(BF))                    # [128, T]
    sinS = sin.T.copy()
    sinS[:64, :] *= -1.0
    sinS = np.ascontiguousarray(sinS.astype(BF))

    # fixed triangular+global mask for the b-4 key block, [k_off, q_off]
    oi = np.arange(128)
    mTm = ((oi[None, :] <= oi[:, None]) | (oi[:, None] % 64 == 0)).astype(BF)

    # global-column mask [g, q]: visible iff 64 g < 128 (q//128 - 4)
    g = np.arange(NG)
    qb = np.arange(T) // BLOCK
    mGm = (64 * g[:, None] < 128 * (qb[None, :] - 4)).astype(BF)

    onesm = np.ones((128, 128), BF)
    ident = np.eye(128, dtype=BF)

    wk_f = (w_kv_down.astype(np.float32) @ w_k_up.astype(np.float32))  # [C, KVH*HD]
    wv_f = (w_kv_down.astype(np.float32) @ w_v_up.astype(np.float32))

    in_maps = []
    for c in range(N_CORES):
        h0 = 2 * c
        kv = h0 // (H // KVH)
        wq_c = np.ascontiguousarray(
            w_q[:, h0 * HD:(h0 + 2) * HD].astype(BF))
        wk_c = np.ascontiguousarray(
            wk_f[:, kv * HD:(kv + 1) * HD].astype(BF))
        wv_c = np.ascontiguousarray(
            wv_f[:, kv * HD:(kv + 1) * HD].astype(BF))
        wo_c = np.ascontiguousarray(
            w_o[h0 * HD:(h0 + 2) * HD, :].astype(BF))
        in_maps.append({
            "xt": xt, "wq": wq_c, "wk": wk_c, "wv": wv_c, "wo": wo_c,
            "cosd": cosT, "sind": sinS, "maskt": mTm, "maskg": mGm,
            "onesd": onesm, "identd": ident,
        })
    return in_maps


def _get_module():
    if "nc" not in _CACHE:
        _CACHE["nc"] = _build_module()
    return _CACHE["nc"]


def kernel(x, w_q, w_kv_down, w_k_up, w_v_up, w_o):
    from concourse.bass_utils import run_bass_kernel_spmd

    nc = _get_module()
    in_maps = _host_inputs(x, w_q, w_kv_down, w_k_up, w_v_up, w_o)
    res = run_bass_kernel_spmd(nc, in_maps, list(range(N_CORES)))
    acc = np.zeros((T, C), np.float32)
    for c in range(N_CORES):
        acc += np.asarray(res.results[c]["out"], dtype=np.float32)
    return acc.reshape(1, T, C)


# revision 3
# speedup vs baseline: 1.0957x; 1.0877x over previous
"""Trainium2 Bass kernel for block-causal sparse attention (MLA-style KV).

Sharding: tensor-parallel over heads. 16 heads / 8 cores = 2 heads per core,
one KV head per core-pair. Each core computes q/k/v projections from the full
(transposed) x, RoPE, sparse attention for its 2 heads, and a partial output
projection; the host sums the 8 partial outputs.

Sparsity structure (T=4096, BLOCK=128, WINDOW=512, GLOBAL_EVERY=64):
for query block b, visible keys are blocks b-4..b (block b-4 masked by a fixed
triangular+global pattern) plus "global" columns j%64==0 with j < 128*(b-4).

All matmul operands are bf16 (fp32 PSUM accumulation). Scores are computed
transposed ([k, q] layout) so probabilities feed the PV and output-projection
matmuls with no transposes. Softmax denominators are accumulated with
ones-matrix matmuls directly in broadcast form ([128, q] in PSUM), so the
reciprocal feeds the normalize multiply without a partition broadcast.

The PE stream is kept dense by interleaving the previous tile's output
projection ("wo filler" pairs) into the stall points of the current tile's
pipeline: after each projection pass (while RoPE chains run on ACT/DVE) and
between attention items (while the exp chain runs on ACT). DMAs are
consolidated (x in 4 chunks/tile, single-shot weights, full-T rope tables,
row-batched output) to keep HWDGE holds off the critical path.
"""

import numpy as np

N_CORES = 8
T = 4096
C = 2048
L = 512
H = 16
KVH = 4
HD = 128
BLOCK = 128
WINDOW = 512
GLOBAL_EVERY = 64
ROPE_THETA = 10000.0

QTW = 512            # query tile width (4 blocks)
NQT = T // QTW       # 8
NKT = C // 128       # 16 contraction tiles for projections
NG = T // GLOBAL_EVERY  # 64 global columns

_CACHE = {}


def _build_module():
    import concourse.bacc as bacc
    import concourse.mybir as mybir
    import concourse.tile as tile
    from contextlib import ExitStack

    F32 = mybir.dt.float32
    BF16 = mybir.dt.bfloat16
    EXP = mybir.ActivationFunctionType.Exp

    nc = bacc.Bacc("TRN2", target_bir_lowering=False, debug=False,
                   num_devices=N_CORES)

    xt = nc.dram_tensor("xt", [C, T], BF16, kind="ExternalInput")
    wq = nc.dram_tensor("wq", [C, 2 * HD], BF16, kind="ExternalInput")
    wk = nc.dram_tensor("wk", [C, HD], BF16, kind="ExternalInput")
    wv = nc.dram_tensor("wv", [C, HD], BF16, kind="ExternalInput")
    wo = nc.dram_tensor("wo", [2 * HD, C], BF16, kind="ExternalInput")
    cosd = nc.dram_tensor("cosd", [HD, T], BF16, kind="ExternalInput")
    sind = nc.dram_tensor("sind", [HD, T], BF16, kind="ExternalInput")  # sign-folded
    maskt = nc.dram_tensor("maskt", [128, 128], BF16, kind="ExternalInput")
    maskg = nc.dram_tensor("maskg", [NG, T], BF16, kind="ExternalInput")
    onesd = nc.dram_tensor("onesd", [128, 128], BF16, kind="ExternalInput")
    identd = nc.dram_tensor("identd", [128, 128], BF16, kind="ExternalInput")
    out = nc.dram_tensor("out", [T, C], BF16, kind="ExternalOutput")

    scale = 1.0 / np.sqrt(HD)

    with tile.TileContext(nc) as tc, ExitStack() as ctx:
        res = ctx.enter_context(tc.tile_pool(name="res", bufs=1))
        kT = res.tile([128, T], BF16, tag="kT")
        vN = res.tile([128, T], BF16, tag="vN")
        kG = res.tile([128, NG], BF16, tag="kG")
        vG = res.tile([64, 128], BF16, tag="vG")
        vGT = res.tile([128, NG], BF16, tag="vGT")
        mT = res.tile([128, 128], BF16, tag="mT")
        mG = res.tile([NG, T], BF16, tag="mG")
        ones = res.tile([128, 128], BF16, tag="ones")
        ident = res.tile([128, 128], BF16, tag="ident")
        wo_sb = res.tile([128, 2 * C], BF16, tag="wo_sb")
        wq_sb = res.tile([128, NKT * 256], BF16, tag="wq_sb")
        wk_sb = res.tile([128, NKT * 128], BF16, tag="wk_sb")
        wv_sb = res.tile([128, NKT * 128], BF16, tag="wv_sb")
        cosF = res.tile([128, T], BF16, tag="cosF")
        sinF = res.tile([128, T], BF16, tag="sinF")

        xpool = ctx.enter_context(tc.tile_pool(name="xpool", bufs=3))
        qlp = ctx.enter_context(tc.tile_pool(name="qlp", bufs=2))
        vtp = ctx.enter_context(tc.tile_pool(name="vtp", bufs=2))
        swp = ctx.enter_context(tc.tile_pool(name="swp", bufs=2))
        tmpp = ctx.enter_context(tc.tile_pool(name="tmpp", bufs=2))
        ppool = ctx.enter_context(tc.tile_pool(name="ppool", bufs=3))
        ynp = ctx.enter_context(tc.tile_pool(name="ynp", bufs=2))
        recp = ctx.enter_context(tc.tile_pool(name="recp", bufs=2))
        obp = ctx.enter_context(tc.tile_pool(name="obp", bufs=2))

        pjps = ctx.enter_context(tc.tile_pool(name="pjps", bufs=2, space="PSUM"))
        spool = ctx.enter_context(tc.tile_pool(name="spool", bufs=2, space="PSUM"))
        ypool = ctx.enter_context(tc.tile_pool(name="ypool", bufs=1, space="PSUM"))
        dpool = ctx.enter_context(tc.tile_pool(name="dpool", bufs=1, space="PSUM"))
        opool = ctx.enter_context(tc.tile_pool(name="opool", bufs=2, space="PSUM"))

        # ---- deferred output-projection "filler" steps ------------------
        # Each step emits the 2-matmul PSUM pair for one (qs, n) output tile
        # plus its PSUM->SBUF copy and (once a row is complete) the DMA.
        wo_state = {"steps": [], "idx": 0}

        def make_wo_steps(ynorm, qs0, last=False):
            steps = []
            obs = {}

            def step(qs, n):
                def run():
                    if n == 0:
                        obs[qs] = obp.tile([128, 2048], BF16, tag="ob", name="ob")
                    o_ps = opool.tile([128, 512], F32, tag="o", name="o_ps")
                    nc.tensor.matmul(o_ps[:], ynorm[0][:, qs * 128:(qs + 1) * 128],
                                     wo_sb[:, n * 512:n * 512 + 512],
                                     start=True, stop=False)
                    nc.tensor.matmul(o_ps[:], ynorm[1][:, qs * 128:(qs + 1) * 128],
                                     wo_sb[:, C + n * 512:C + n * 512 + 512],
                                     start=False, stop=True)
                    ob = obs[qs]
                    if (qs * 4 + n) % 2 == 0:
                        nc.scalar.copy(ob[:, n * 512:(n + 1) * 512], o_ps[:])
                    else:
                        nc.vector.tensor_copy(ob[:, n * 512:(n + 1) * 512], o_ps[:])
                    if n == 3:
                        rows = slice(qs0 + qs * 128, qs0 + (qs + 1) * 128)
                        nc.sync.dma_start(out[rows, :], ob[:])
                return run

            for qs in range(4):
                for n in range(4):
                    steps.append(step(qs, n))
            return steps

        def fill(n):
            st = wo_state
            while n > 0 and st["idx"] < len(st["steps"]):
                st["steps"][st["idx"]]()
                st["idx"] += 1
                n -= 1

        def fill_all():
            fill(len(wo_state["steps"]))

        for it in range(NQT):
            nt = it
            b0 = 4 * it
            ts = slice(nt * 512, (nt + 1) * 512)
            qs0 = it * QTW

            # ---- x / weight DMAs (consolidated) ----
            if it == 0:
                nc.sync.dma_start(
                    wq_sb[:].rearrange("p (a d) -> p a d", a=NKT),
                    wq[:, :].rearrange("(a p) d -> p a d", p=128))
                nc.gpsimd.dma_start(ident[:], identd[:])
                nc.gpsimd.dma_start(mT[:], maskt[:])
                nc.gpsimd.dma_start(ones[:], onesd[:])
                nc.sync.dma_start(cosF[:], cosd[:, :])
                nc.sync.dma_start(sinF[:], sind[:, :])
            xbig = xpool.tile([128, NKT * 512], BF16, tag="xtile")
            for q4 in range(4):
                nc.sync.dma_start(
                    xbig[:, q4 * 2048:(q4 + 1) * 2048].rearrange(
                        "p (a t) -> p a t", a=4),
                    xt[q4 * 512:(q4 + 1) * 512, ts].rearrange(
                        "(a p) t -> p a t", p=128))
            xts = [xbig[:, kt * 512:(kt + 1) * 512] for kt in range(NKT)]
            if it == 0:
                nc.sync.dma_start(
                    wk_sb[:].rearrange("p (a d) -> p a d", a=NKT),
                    wk[:, :].rearrange("(a p) d -> p a d", p=128))
                nc.sync.dma_start(
                    wv_sb[:].rearrange("p (a d) -> p a d", a=NKT),
                    wv[:, :].rearrange("(a p) d -> p a d", p=128))

            cos_t = cosF[:, ts]
            sin_t = sinF[:, ts]

            qloc = [qlp.tile([128, 512], BF16, tag=f"ql{h}", name=f"ql{h}")
                    for h in range(2)]
            # pass order: k, v first so the RoPE/transpose chains complete
            # while the q passes still feed the PE
            wslices = [
                lambda kt: wk_sb[:, kt * 128:(kt + 1) * 128],
                lambda kt: wv_sb[:, kt * 128:(kt + 1) * 128],
                lambda kt: wq_sb[:, kt * 256:kt * 256 + 128],
                lambda kt: wq_sb[:, kt * 256 + 128:kt * 256 + 256],
            ]
            ropedest = [kT[:, ts], None, None, None]
            vT_t = vtp.tile([128, 512], BF16, tag="vT")
            for i in range(4):
                pj = pjps.tile([128, 512], F32, tag="pj")
                for kt in range(NKT):
                    nc.tensor.matmul(pj[:], wslices[i](kt), xts[kt][:],
                                     start=(kt == 0), stop=(kt == NKT - 1))
                if i != 1:
                    # RoPE: dest = qsb*cos + swap(qsb)*sinS
                    dest = kT[:, ts] if i == 0 else qloc[i - 2][:]
                    qsb = swp.tile([128, 512], BF16, tag="qsb")
                    nc.scalar.copy(qsb[:], pj[:])
                    sw = swp.tile([128, 512], BF16, tag="sw")
                    nc.gpsimd.dma_start(sw[0:64, :], qsb[64:128, :])
                    nc.gpsimd.dma_start(sw[64:128, :], qsb[0:64, :])
                    ta = tmpp.tile([128, 512], BF16, tag="ta")
                    nc.vector.tensor_mul(ta[:], qsb[:], cos_t)
                    tb = tmpp.tile([128, 512], BF16, tag="tb")
                    nc.vector.tensor_mul(tb[:], sw[:], sin_t)
                    nc.vector.tensor_add(dest, ta[:], tb[:])
                else:
                    nc.vector.tensor_copy(vT_t[:], pj[:])
                fill(2)

            if it == 0:
                nc.gpsimd.dma_start(mG[:], maskg[:])
                for i in range(2):
                    nc.sync.dma_start(wo_sb[:, i * C:(i + 1) * C],
                                      wo[i * 128:(i + 1) * 128, :])

            # ---- v transpose for this t-tile + incremental global K/V ----
            fill(2)
            for j in range(4):
                blk = nt * 4 + j
                tp = spool.tile([128, 512], BF16, tag="s", name="tp")
                nc.tensor.transpose(tp[:, :128], vT_t[:, j * 128:(j + 1) * 128],
                                    ident[:])
                nc.vector.tensor_copy(vN[:, blk * 128:(blk + 1) * 128], tp[:, :128])
            gsl = slice(nt * 8, (nt + 1) * 8)
            nc.vector.tensor_copy(kG[:, gsl], kT[:, ts][:, 0:512:GLOBAL_EVERY])
            nc.vector.tensor_copy(vGT[:, gsl], vT_t[:][:, 0:512:GLOBAL_EVERY])
            gw2 = 8 * (nt + 1)
            tpg = spool.tile([128, 512], BF16, tag="s", name="tpg")
            nc.tensor.transpose(tpg[:gw2, :128], vGT[:, :gw2], ident[:])
            nc.vector.tensor_copy(vG[:gw2, :], tpg[:gw2, :128])

            # ---- attention for query tile `it` (4 blocks b0..b0+3) ----
            gw = min(NG, 8 * it)   # written prefix of kG/vG; 0 for it=0
            ynorm = []
            for h in range(2):
                items = [(b0, 0, 512, None)]
                if it == 0:
                    for j in range(3):
                        items.append((j + 1, (j + 1) * 128, (3 - j) * 128, None))
                    use_glob = False
                else:
                    for j in range(4):
                        items.append((b0 - 4 + j, 0, (j + 1) * 128, j))
                    for j in range(3):
                        items.append((b0 + 1 + j, (j + 1) * 128, (3 - j) * 128, None))
                    use_glob = gw > 0

                y_ps = ypool.tile([128, QTW], F32, tag="y")
                d_ps = dpool.tile([128, QTW], F32, tag="d")
                n_items = len(items) + (1 if use_glob else 0)
                s_tiles = [None] * n_items

                def emit_qk(ii):
                    s = spool.tile([128, QTW], F32, tag="s")
                    if ii < len(items):
                        kb, qoff, w, _ = items[ii]
                        nc.tensor.matmul(
                            s[:, :w], kT[:, kb * 128:(kb + 1) * 128],
                            qloc[h][:, qoff:qoff + w],
                            start=True, stop=True)
                    else:
                        nc.tensor.matmul(s[:gw, :], kG[:, :gw], qloc[h][:],
                                         start=True, stop=True)
                    s_tiles[ii] = s

                def emit_rest(ii):
                    first = ii == 0
                    last = ii == n_items - 1
                    s = s_tiles[ii]
                    p = ppool.tile([128, QTW], BF16, tag="p")
                    if ii < len(items):
                        kb, qoff, w, tri = items[ii]
                        nc.scalar.activation(p[:, :w], s[:, :w], EXP, scale=scale)
                        if tri is not None:
                            nc.vector.tensor_mul(p[:, tri * 128:(tri + 1) * 128],
                                                 p[:, tri * 128:(tri + 1) * 128],
                                                 mT[:])
                        nc.tensor.matmul(y_ps[:, qoff:qoff + w],
                                         vN[:, kb * 128:(kb + 1) * 128], p[:, :w],
                                         start=first, stop=last)
                        nc.tensor.matmul(d_ps[:, qoff:qoff + w], ones[:, :],
                                         p[:, :w], start=first, stop=last)
                    else:
                        nc.scalar.activation(p[:gw, :], s[:gw, :], EXP, scale=scale)
                        nc.vector.tensor_mul(p[:gw, :], p[:gw, :],
                                             mG[:gw, qs0:qs0 + QTW])
                        nc.tensor.matmul(y_ps[:, :], vG[:gw, :], p[:gw, :],
                                         start=first, stop=last)
                        nc.tensor.matmul(d_ps[:, :], ones[:gw, :], p[:gw, :],
                                         start=first, stop=last)

                emit_qk(0)
                for ii in range(n_items):
                    if ii + 1 < n_items:
                        emit_qk(ii + 1)
                    fill(1)
                    emit_rest(ii)

                # d_ps holds the denominator replicated across partitions, so
                # the reciprocal is already in broadcast form for the multiply
                rbc = recp.tile([128, QTW], F32, tag="rbc")
                nc.vector.reciprocal(rbc[:], d_ps[:])
                yn = ynp.tile([128, QTW], BF16, tag=f"yn{h}", name=f"yn{h}")
                nc.vector.tensor_mul(yn[:], y_ps[:], rbc[:])
                ynorm.append(yn)
                fill(2)

            # ---- output projection: deferred as filler for the next
            # iteration's stall points ----
            fill_all()
            wo_state = {"steps": make_wo_steps(ynorm, qs0, last=(it == NQT - 1)),
                        "idx": 0}

        fill_all()

    nc.compile()
    return nc


def _host_inputs(x, w_q, w_kv_down, w_k_up, w_v_up, w_o):
    """Build the per-core input maps (host-side shard + precompute)."""
    import ml_dtypes
    BF = ml_dtypes.bfloat16
    x = np.asarray(x)
    w_q = np.asarray(w_q)
    w_kv_down = np.asarray(w_kv_down)
    w_k_up = np.asarray(w_k_up)
    w_v_up = np.asarray(w_v_up)
    w_o = np.asarray(w_o)
    x2 = np.ascontiguousarray(x.reshape(T, C).astype(np.float32))
    xt = np.ascontiguousarray(x2.T.astype(BF))

    # RoPE tables, [hd, t] layout, sign folded into sin for the swapped term
    freqs = 1.0 / (ROPE_THETA ** (np.arange(0, HD, 2, dtype=np.float64) / HD))
    emb = np.arange(T, dtype=np.float64)[:, None] * freqs[None, :]   # [T, 64]
    cos = np.concatenate([np.cos(emb), np.cos(emb)], axis=-1)        # [T, 128]
    sin = np.concatenate([np.sin(emb), np.sin(emb)], axis=-1)
    cosT = np.ascontiguousarray(cos.T.astype(BF))                    # [128, T]
    sinS = sin.T.copy()
    sinS[:64, :] *= -1.0
    sinS = np.ascontiguousarray(sinS.astype(BF))

    # fixed triangular+global mask for the b-4 key block, [k_off, q_off]
    oi = np.arange(128)
    mTm = ((oi[None, :] <= oi[:, None]) | (oi[:, None] % 64 == 0)).astype(BF)

    # global-column mask [g, q]: visible iff 64 g < 128 (q//128 - 4)
    g = np.arange(NG)
    qb = np.arange(T) // BLOCK
    mGm = (64 * g[:, None] < 128 * (qb[None, :] - 4)).astype(BF)

    onesm = np.ones((128, 128), BF)
    ident = np.eye(128, dtype=BF)

    wk_f = (w_kv_down.astype(np.float32) @ w_k_up.astype(np.float32))  # [C, KVH*HD]
    wv_f = (w_kv_down.astype(np.float32) @ w_v_up.astype(np.float32))

    in_maps = []
    for c in range(N_CORES):
        h0 = 2 * c
        kv = h0 // (H // KVH)
        wq_c = np.ascontiguousarray(
            w_q[:, h0 * HD:(h0 + 2) * HD].astype(BF))
        wk_c = np.ascontiguousarray(
            wk_f[:, kv * HD:(kv + 1) * HD].astype(BF))
        wv_c = np.ascontiguousarray(
            wv_f[:, kv * HD:(kv + 1) * HD].astype(BF))
        wo_c = np.ascontiguousarray(
            w_o[h0 * HD:(h0 + 2) * HD, :].astype(BF))
        in_maps.append({
            "xt": xt, "wq": wq_c, "wk": wk_c, "wv": wv_c, "wo": wo_c,
            "cosd": cosT, "sind": sinS, "maskt": mTm, "maskg": mGm,
            "onesd": onesm, "identd": ident,
        })
    return in_maps


def _get_module():
    if "nc" not in _CACHE:
        _CACHE["nc"] = _build_module()
    return _CACHE["nc"]


def kernel(x, w_q, w_kv_down, w_k_up, w_v_up, w_o):
    from concourse.bass_utils import run_bass_kernel_spmd

    nc = _get_module()
    in_maps = _host_inputs(x, w_q, w_kv_down, w_k_up, w_v_up, w_o)
    res = run_bass_kernel_spmd(nc, in_maps, list(range(N_CORES)))
    acc = np.zeros((T, C), np.float32)
    for c in range(N_CORES):
        acc += np.asarray(res.results[c]["out"], dtype=np.float32)
    return acc.reshape(1, T, C)


# revision 5
# speedup vs baseline: 1.0961x; 1.0003x over previous
"""Trainium2 Bass kernel for block-causal sparse attention (MLA-style KV).

Sharding: tensor-parallel over heads. 16 heads / 8 cores = 2 heads per core,
one KV head per core-pair. Each core computes q/k/v projections from the full
(transposed) x, RoPE, sparse attention for its 2 heads, and a partial output
projection; the host sums the 8 partial outputs.

Sparsity structure (T=4096, BLOCK=128, WINDOW=512, GLOBAL_EVERY=64):
for query block b, visible keys are blocks b-4..b (block b-4 masked by a fixed
triangular+global pattern) plus "global" columns j%64==0 with j < 128*(b-4).

All matmul operands are bf16 (fp32 PSUM accumulation). Scores are computed
transposed ([k, q] layout) so probabilities feed the PV and output-projection
matmuls with no transposes. Softmax denominators are accumulated with
ones-matrix matmuls directly in broadcast form ([128, q] in PSUM), so the
reciprocal feeds the normalize multiply without a partition broadcast.

The PE stream is kept dense by interleaving the previous tile's output
projection ("wo filler" pairs) into the stall points of the current tile's
pipeline: after each projection pass (while RoPE chains run on ACT/DVE) and
between attention items (while the exp chain runs on ACT). DMAs are
consolidated (x in 4 chunks/tile, single-shot weights, full-T rope tables,
row-batched output) to keep HWDGE holds off the critical path.
"""

import numpy as np

N_CORES = 8
T = 4096
C = 2048
L = 512
H = 16
KVH = 4
HD = 128
BLOCK = 128
WINDOW = 512
GLOBAL_EVERY = 64
ROPE_THETA = 10000.0

QTW = 512            # query tile width (4 blocks)
NQT = T // QTW       # 8
NKT = C // 128       # 16 contraction tiles for projections
NG = T // GLOBAL_EVERY  # 64 global columns

_CACHE = {}


def _build_module():
    import concourse.bacc as bacc
    import concourse.mybir as mybir
    import concourse.tile as tile
    from contextlib import ExitStack

    F32 = mybir.dt.float32
    BF16 = mybir.dt.bfloat16
    EXP = mybir.ActivationFunctionType.Exp

    nc = bacc.Bacc("TRN2", target_bir_lowering=False, debug=False,
                   num_devices=N_CORES)

    xt = nc.dram_tensor("xt", [C, T], BF16, kind="ExternalInput")
    wq = nc.dram_tensor("wq", [C, 2 * HD], BF16, kind="ExternalInput")
    wk = nc.dram_tensor("wk", [C, HD], BF16, kind="ExternalInput")
    wv = nc.dram_tensor("wv", [C, HD], BF16, kind="ExternalInput")
    wo = nc.dram_tensor("wo", [2 * HD, C], BF16, kind="ExternalInput")
    cosd = nc.dram_tensor("cosd", [HD, T], BF16, kind="ExternalInput")
    sind = nc.dram_tensor("sind", [HD, T], BF16, kind="ExternalInput")  # sign-folded
    maskt = nc.dram_tensor("maskt", [128, 128], BF16, kind="ExternalInput")
    maskg = nc.dram_tensor("maskg", [NG, T], BF16, kind="ExternalInput")
    onesd = nc.dram_tensor("onesd", [128, 128], BF16, kind="ExternalInput")
    identd = nc.dram_tensor("identd", [128, 128], BF16, kind="ExternalInput")
    out = nc.dram_tensor("out", [T, C], BF16, kind="ExternalOutput")

    scale = 1.0 / np.sqrt(HD)

    with tile.TileContext(nc) as tc, ExitStack() as ctx:
        res = ctx.enter_context(tc.tile_pool(name="res", bufs=1))
        kT = res.tile([128, T], BF16, tag="kT")
        vN = res.tile([128, T], BF16, tag="vN")
        kG = res.tile([128, NG], BF16, tag="kG")
        vG = res.tile([64, 128], BF16, tag="vG")
        vGT = res.tile([128, NG], BF16, tag="vGT")
        mT = res.tile([128, 128], BF16, tag="mT")
        mG = res.tile([NG, T], BF16, tag="mG")
        ones = res.tile([128, 128], BF16, tag="ones")
        ident = res.tile([128, 128], BF16, tag="ident")
        wo_sb = res.tile([128, 2 * C], BF16, tag="wo_sb")
        wq_sb = res.tile([128, NKT * 256], BF16, tag="wq_sb")
        wk_sb = res.tile([128, NKT * 128], BF16, tag="wk_sb")
        wv_sb = res.tile([128, NKT * 128], BF16, tag="wv_sb")
        cosF = res.tile([128, T], BF16, tag="cosF")
        sinF = res.tile([128, T], BF16, tag="sinF")

        xpool = ctx.enter_context(tc.tile_pool(name="xpool", bufs=3))
        qlp = ctx.enter_context(tc.tile_pool(name="qlp", bufs=2))
        vtp = ctx.enter_context(tc.tile_pool(name="vtp", bufs=2))
        swp = ctx.enter_context(tc.tile_pool(name="swp", bufs=2))
        tmpp = ctx.enter_context(tc.tile_pool(name="tmpp", bufs=2))
        ppool = ctx.enter_context(tc.tile_pool(name="ppool", bufs=3))
        ynp = ctx.enter_context(tc.tile_pool(name="ynp", bufs=2))
        recp = ctx.enter_context(tc.tile_pool(name="recp", bufs=2))
        obp = ctx.enter_context(tc.tile_pool(name="obp", bufs=2))

        pjps = ctx.enter_context(tc.tile_pool(name="pjps", bufs=2, space="PSUM"))
        spool = ctx.enter_context(tc.tile_pool(name="spool", bufs=2, space="PSUM"))
        ypool = ctx.enter_context(tc.tile_pool(name="ypool", bufs=1, space="PSUM"))
        dpool = ctx.enter_context(tc.tile_pool(name="dpool", bufs=1, space="PSUM"))
        opool = ctx.enter_context(tc.tile_pool(name="opool", bufs=2, space="PSUM"))

        # ---- deferred output-projection "filler" steps ------------------
        # Each step emits the 2-matmul PSUM pair for one (qs, n) output tile
        # plus its PSUM->SBUF copy and (once a row is complete) the DMA.
        wo_state = {"steps": [], "idx": 0}

        def make_wo_steps(ynorm, qs0, last=False):
            steps = []
            obs = {}

            def step(qs, n):
                def run():
                    if n == 0:
                        obs[qs] = obp.tile([128, 2048], BF16, tag="ob", name="ob")
                    o_ps = opool.tile([128, 512], F32, tag="o", name="o_ps")
                    nc.tensor.matmul(o_ps[:], ynorm[0][:, qs * 128:(qs + 1) * 128],
                                     wo_sb[:, n * 512:n * 512 + 512],
                                     start=True, stop=False)
                    nc.tensor.matmul(o_ps[:], ynorm[1][:, qs * 128:(qs + 1) * 128],
                                     wo_sb[:, C + n * 512:C + n * 512 + 512],
                                     start=False, stop=True)
                    ob = obs[qs]
                    if (qs * 4 + n) % 2 == 0:
                        nc.scalar.copy(ob[:, n * 512:(n + 1) * 512], o_ps[:])
                    else:
                        nc.vector.tensor_copy(ob[:, n * 512:(n + 1) * 512], o_ps[:])
                    if n == 3:
                        rows = slice(qs0 + qs * 128, qs0 + (qs + 1) * 128)
                        nc.sync.dma_start(out[rows, :], ob[:])
                return run

            for qs in range(4):
                for n in range(4):
                    steps.append(step(qs, n))
            return steps

        def fill(n):
            st = wo_state
            while n > 0 and st["idx"] < len(st["steps"]):
                st["steps"][st["idx"]]()
                st["idx"] += 1
                n -= 1

        def fill_all():
            fill(len(wo_state["steps"]))

        for it in range(NQT):
            nt = it
            b0 = 4 * it
            ts = slice(nt * 512, (nt + 1) * 512)
            qs0 = it * QTW

            # ---- x / weight DMAs (consolidated; k-weights first so the
            # first projection pass can start as soon as x chunk 0 lands) ----
            if it == 0:
                nc.sync.dma_start(
                    wk_sb[:].rearrange("p (a d) -> p a d", a=NKT),
                    wk[:, :].rearrange("(a p) d -> p a d", p=128))
                nc.gpsimd.dma_start(ident[:], identd[:])
                nc.gpsimd.dma_start(mT[:], maskt[:])
                nc.gpsimd.dma_start(ones[:], onesd[:])
            xbig = xpool.tile([128, NKT * 512], BF16, tag="xtile")
            for q4 in range(4):
                nc.sync.dma_start(
                    xbig[:, q4 * 2048:(q4 + 1) * 2048].rearrange(
                        "p (a t) -> p a t", a=4),
                    xt[q4 * 512:(q4 + 1) * 512, ts].rearrange(
                        "(a p) t -> p a t", p=128))
            xts = [xbig[:, kt * 512:(kt + 1) * 512] for kt in range(NKT)]
            if it == 0:
                nc.sync.dma_start(
                    wq_sb[:].rearrange("p (a d) -> p a d", a=NKT),
                    wq[:, :].rearrange("(a p) d -> p a d", p=128))
                nc.sync.dma_start(
                    wv_sb[:].rearrange("p (a d) -> p a d", a=NKT),
                    wv[:, :].rearrange("(a p) d -> p a d", p=128))
                nc.sync.dma_start(cosF[:], cosd[:, :])
                nc.sync.dma_start(sinF[:], sind[:, :])

            cos_t = cosF[:, ts]
            sin_t = sinF[:, ts]

            qloc = [qlp.tile([128, 512], BF16, tag=f"ql{h}", name=f"ql{h}")
                    for h in range(2)]
            # pass order k, q0, v, q1: each RoPE chain hides under the
            # following projection passes so kT/qloc are ready for attention
            wslices = [
                lambda kt: wk_sb[:, kt * 128:(kt + 1) * 128],
                lambda kt: wq_sb[:, kt * 256:kt * 256 + 128],
                lambda kt: wv_sb[:, kt * 128:(kt + 1) * 128],
                lambda kt: wq_sb[:, kt * 256 + 128:kt * 256 + 256],
            ]
            vT_t = vtp.tile([128, 512], BF16, tag="vT")
            ropedest = [kT[:, ts], qloc[0][:], None, qloc[1][:]]
            for i in range(4):
                pj = pjps.tile([128, 512], F32, tag="pj")
                for kt in range(NKT):
                    nc.tensor.matmul(pj[:], wslices[i](kt), xts[kt][:],
                                     start=(kt == 0), stop=(kt == NKT - 1))
                if i != 2:
                    # RoPE: dest = qsb*cos + swap(qsb)*sinS
                    dest = ropedest[i]
                    qsb = swp.tile([128, 512], BF16, tag="qsb")
                    nc.scalar.copy(qsb[:], pj[:])
                    sw = swp.tile([128, 512], BF16, tag="sw")
                    nc.gpsimd.dma_start(sw[0:64, :], qsb[64:128, :])
                    nc.gpsimd.dma_start(sw[64:128, :], qsb[0:64, :])
                    ta = tmpp.tile([128, 512], BF16, tag="ta")
                    nc.vector.tensor_mul(ta[:], qsb[:], cos_t)
                    tb = tmpp.tile([128, 512], BF16, tag="tb")
                    nc.vector.tensor_mul(tb[:], sw[:], sin_t)
                    nc.vector.tensor_add(dest, ta[:], tb[:])
                else:
                    nc.vector.tensor_copy(vT_t[:], pj[:])
                fill(2)

            if it == 0:
                nc.gpsimd.dma_start(mG[:], maskg[:])
                for i in range(2):
                    nc.sync.dma_start(wo_sb[:, i * C:(i + 1) * C],
                                      wo[i * 128:(i + 1) * 128, :])

            # ---- v transpose for this t-tile + incremental global K/V ----
            fill(2)
            for j in range(4):
                blk = nt * 4 + j
                tp = spool.tile([128, 512], BF16, tag="s", name="tp")
                nc.tensor.transpose(tp[:, :128], vT_t[:, j * 128:(j + 1) * 128],
                                    ident[:])
                nc.vector.tensor_copy(vN[:, blk * 128:(blk + 1) * 128], tp[:, :128])
            gsl = slice(nt * 8, (nt + 1) * 8)
            nc.vector.tensor_copy(kG[:, gsl], kT[:, ts][:, 0:512:GLOBAL_EVERY])
            nc.vector.tensor_copy(vGT[:, gsl], vT_t[:][:, 0:512:GLOBAL_EVERY])
            gw2 = 8 * (nt + 1)
            tpg = spool.tile([128, 512], BF16, tag="s", name="tpg")
            nc.tensor.transpose(tpg[:gw2, :128], vGT[:, :gw2], ident[:])
            nc.vector.tensor_copy(vG[:gw2, :], tpg[:gw2, :128])

            # ---- attention for query tile `it` (4 blocks b0..b0+3) ----
            gw = min(NG, 8 * it)   # written prefix of kG/vG; 0 for it=0
            ynorm = []
            for h in range(2):
                items = [(b0, 0, 512, None)]
                if it == 0:
                    for j in range(3):
                        items.append((j + 1, (j + 1) * 128, (3 - j) * 128, None))
                    use_glob = False
                else:
                    for j in range(4):
                        items.append((b0 - 4 + j, 0, (j + 1) * 128, j))
                    for j in range(3):
                        items.append((b0 + 1 + j, (j + 1) * 128, (3 - j) * 128, None))
                    use_glob = gw > 0

                y_ps = ypool.tile([128, QTW], F32, tag="y")
                d_ps = dpool.tile([128, QTW], F32, tag="d")
                n_items = len(items) + (1 if use_glob else 0)
                s_tiles = [None] * n_items

                def emit_qk(ii):
                    s = spool.tile([128, QTW], F32, tag="s")
                    if ii < len(items):
                        kb, qoff, w, _ = items[ii]
                        nc.tensor.matmul(
                            s[:, :w], kT[:, kb * 128:(kb + 1) * 128],
                            qloc[h][:, qoff:qoff + w],
                            start=True, stop=True)
                    else:
                        nc.tensor.matmul(s[:gw, :], kG[:, :gw], qloc[h][:],
                                         start=True, stop=True)
                    s_tiles[ii] = s

                def emit_rest(ii):
                    first = ii == 0
                    last = ii == n_items - 1
                    s = s_tiles[ii]
                    p = ppool.tile([128, QTW], BF16, tag="p")
                    if ii < len(items):
                        kb, qoff, w, tri = items[ii]
                        nc.scalar.activation(p[:, :w], s[:, :w], EXP, scale=scale)
                        if tri is not None:
                            nc.vector.tensor_mul(p[:, tri * 128:(tri + 1) * 128],
                                                 p[:, tri * 128:(tri + 1) * 128],
                                                 mT[:])
                        nc.tensor.matmul(y_ps[:, qoff:qoff + w],
                                         vN[:, kb * 128:(kb + 1) * 128], p[:, :w],
                                         start=first, stop=last)
                        nc.tensor.matmul(d_ps[:, qoff:qoff + w], ones[:, :],
                                         p[:, :w], start=first, stop=last)
                    else:
                        nc.scalar.activation(p[:gw, :], s[:gw, :], EXP, scale=scale)
                        nc.vector.tensor_mul(p[:gw, :], p[:gw, :],
                                             mG[:gw, qs0:qs0 + QTW])
                        nc.tensor.matmul(y_ps[:, :], vG[:gw, :], p[:gw, :],
                                         start=first, stop=last)
                        nc.tensor.matmul(d_ps[:, :], ones[:gw, :], p[:gw, :],
                                         start=first, stop=last)

                emit_qk(0)
                for ii in range(n_items):
                    if ii + 1 < n_items:
                        emit_qk(ii + 1)
                    fill(1)
                    emit_rest(ii)

                # d_ps holds the denominator replicated across partitions, so
                # the reciprocal is already in broadcast form for the multiply
                rbc = recp.tile([128, QTW], F32, tag="rbc")
                nc.vector.reciprocal(rbc[:], d_ps[:])
                yn = ynp.tile([128, QTW], BF16, tag=f"yn{h}", name=f"yn{h}")
                nc.vector.tensor_mul(yn[:], y_ps[:], rbc[:])
                ynorm.append(yn)
                fill(2)

            # ---- output projection: deferred as filler for the next
            # iteration's stall points ----
            fill_all()
            wo_state = {"steps": make_wo_steps(ynorm, qs0, last=(it == NQT - 1)),
                        "idx": 0}

        fill_all()

    nc.compile()
    return nc


def _host_inputs(x, w_q, w_kv_down, w_k_up, w_v_up, w_o):
    """Build the per-core input maps (host-side shard + precompute)."""
    import ml_dtypes
    BF = ml_dtypes.bfloat16
    x = np.asarray(x)
    w_q = np.asarray(w_q)
    w_kv_down = np.asarray(w_kv_down)
    w_k_up = np.asarray(w_k_up)
    w_v_up = np.asarray(w_v_up)
    w_o = np.asarray(w_o)
    x2 = np.ascontiguousarray(x.reshape(T, C).astype(np.float32))
    xt = np.ascontiguousarray(x2.T.astype(BF))

    # RoPE tables, [hd, t] layout, sign folded into sin for the swapped term
    freqs = 1.0 / (ROPE_THETA ** (np.arange(0, HD, 2, dtype=np.float64) / HD))
    emb = np.arange(T, dtype=np.float64)[:, None] * freqs[None, :]   # [T, 64]
    cos = np.concatenate([np.cos(emb), np.cos(emb)], axis=-1)        # [T, 128]
    sin = np.concatenate([np.sin(emb), np.sin(emb)], axis=-1)
    cosT = np.ascontiguousarray(cos.T.astype(BF))                    # [128, T]
    sinS = sin.T.copy()
    sinS[:64, :] *= -1.0
    sinS = np.ascontiguousarray(sinS.astype(BF))

    # fixed triangular+global mask for the b-4 key block, [k_off, q_off]
    oi = np.arange(128)
    mTm = ((oi[None, :] <= oi[:, None]) | (oi[:, None] % 64 == 0)).astype(BF)

    # global-column mask [g, q]: visible iff 64 g < 128 (q//128 - 4)
    g = np.arange(NG)
    qb = np.arange(T) // BLOCK
    mGm = (64 * g[:, None] < 128 * (qb[None, :] - 4)).astype(BF)

    onesm = np.ones((128, 128), BF)
    ident = np.eye(128, dtype=BF)

    wk_f = (w_kv_down.astype(np.float32) @ w_k_up.astype(np.float32))  # [C, KVH*HD]
    wv_f = (w_kv_down.astype(np.float32) @ w_v_up.astype(np.float32))

    in_maps = []
    for c in range(N_CORES):
        h0 = 2 * c
        kv = h0 // (H // KVH)
        wq_c = np.ascontiguousarray(
            w_q[:, h0 * HD:(h0 + 2) * HD].astype(BF))
        wk_c = np.ascontiguousarray(
            wk_f[:, kv * HD:(kv + 1) * HD].astype(BF))
        wv_c = np.ascontiguousarray(
            wv_f[:, kv * HD:(kv + 1) * HD].astype(BF))
        wo_c = np.ascontiguousarray(
            w_o[h0 * HD:(h0 + 2) * HD, :].astype(BF))
        in_maps.append({
            "xt": xt, "wq": wq_c, "wk": wk_c, "wv": wv_c, "wo": wo_c,
            "cosd": cosT, "sind": sinS, "maskt": mTm, "maskg": mGm,
            "onesd": onesm, "identd": ident,
        })
    return in_maps


def _get_module():
    if "nc" not in _CACHE:
        _CACHE["nc"] = _build_module()
    return _CACHE["nc"]


def kernel(x, w_q, w_kv_down, w_k_up, w_v_up, w_o):
    from concourse.bass_utils import run_bass_kernel_spmd

    nc = _get_module()
    in_maps = _host_inputs(x, w_q, w_kv_down, w_k_up, w_v_up, w_o)
    res = run_bass_kernel_spmd(nc, in_maps, list(range(N_CORES)))
    acc = np.zeros((T, C), np.float32)
    for c in range(N_CORES):
        acc += np.asarray(res.results[c]["out"], dtype=np.float32)
    return acc.reshape(1, T, C)


# revision 11
# speedup vs baseline: 1.2078x; 1.1020x over previous
"""Trainium2 Bass kernel for block-causal sparse attention (MLA-style KV).

Sharding: tensor-parallel over heads. 16 heads / 8 cores = 2 heads per core,
one KV head per core-pair. Each core computes q/k/v projections from the full
(transposed) x, RoPE, sparse attention for its 2 heads, and a partial output
projection; the host sums the 8 partial outputs.

Sparsity structure (T=4096, BLOCK=128, WINDOW=512, GLOBAL_EVERY=64):
for query block b, visible keys are blocks b-4..b (block b-4 masked by a fixed
triangular+global pattern) plus "global" columns j%64==0 with j < 128*(b-4).

All matmul operands are bf16 (fp32 PSUM accumulation). Scores are computed
transposed ([k, q] layout) so probabilities feed the PV and output-projection
matmuls with no transposes. Softmax denominators are accumulated with
ones-matrix matmuls directly in broadcast form ([128, q] in PSUM), so the
reciprocal feeds the normalize multiply without a partition broadcast.

The PE stream is kept dense by interleaving the previous tile's output
projection ("wo filler" pairs) into the stall points of the current tile's
pipeline: after each projection pass (while RoPE chains run on ACT/DVE) and
between attention items (while the exp chain runs on ACT). DMAs are
consolidated (x in 4 chunks/tile, single-shot weights, full-T rope tables,
row-batched output) to keep HWDGE holds off the critical path.
"""

import numpy as np

N_CORES = 8
T = 4096
C = 2048
L = 512
H = 16
KVH = 4
HD = 128
BLOCK = 128
WINDOW = 512
GLOBAL_EVERY = 64
ROPE_THETA = 10000.0

QTW = 512            # query tile width (4 blocks)
NQT = T // QTW       # 8
NKT = C // 128       # 16 contraction tiles for projections
NG = T // GLOBAL_EVERY  # 64 global columns

_CACHE = {}


def _build_module():
    import concourse.bacc as bacc
    import concourse.mybir as mybir
    import concourse.tile as tile
    from contextlib import ExitStack

    F32 = mybir.dt.float32
    BF16 = mybir.dt.bfloat16
    EXP = mybir.ActivationFunctionType.Exp

    nc = bacc.Bacc("TRN2", target_bir_lowering=False, debug=False,
                   num_devices=N_CORES)

    xt = nc.dram_tensor("xt", [C, T], BF16, kind="ExternalInput")
    wq = nc.dram_tensor("wq", [C, 2 * HD], BF16, kind="ExternalInput")
    wk = nc.dram_tensor("wk", [C, HD], BF16, kind="ExternalInput")
    wv = nc.dram_tensor("wv", [C, HD], BF16, kind="ExternalInput")
    wo = nc.dram_tensor("wo", [2 * HD, C], BF16, kind="ExternalInput")
    cosd = nc.dram_tensor("cosd", [HD, T], BF16, kind="ExternalInput")
    sind = nc.dram_tensor("sind", [HD, T], BF16, kind="ExternalInput")  # sign-folded
    maskt = nc.dram_tensor("maskt", [128, 128], BF16, kind="ExternalInput")
    maskg = nc.dram_tensor("maskg", [NG, T], BF16, kind="ExternalInput")
    onesd = nc.dram_tensor("onesd", [128, 128], BF16, kind="ExternalInput")
    identd = nc.dram_tensor("identd", [128, 128], BF16, kind="ExternalInput")
    swapd = nc.dram_tensor("swapd", [128, 128], BF16, kind="ExternalInput")
    out = nc.dram_tensor("out", [T, C], BF16, kind="ExternalOutput")

    scale = 1.0 / np.sqrt(HD)

    with tile.TileContext(nc) as tc, ExitStack() as ctx:
        res = ctx.enter_context(tc.tile_pool(name="res", bufs=1))
        kT = res.tile([128, T], BF16, tag="kT")
        vN = res.tile([128, T], BF16, tag="vN")
        kG = res.tile([128, NG], BF16, tag="kG")
        vG = res.tile([64, 128], BF16, tag="vG")
        vGT = res.tile([128, NG], BF16, tag="vGT")
        mT = res.tile([128, 128], BF16, tag="mT")
        mG = res.tile([NG, T], BF16, tag="mG")
        ones = res.tile([128, 128], BF16, tag="ones")
        ident = res.tile([128, 128], BF16, tag="ident")
        swpm = res.tile([128, 128], BF16, tag="swpm")
        wo_sb = res.tile([128, 2 * C], BF16, tag="wo_sb")
        wq_sb = res.tile([128, NKT * 256], BF16, tag="wq_sb")
        wk_sb = res.tile([128, NKT * 128], BF16, tag="wk_sb")
        wv_sb = res.tile([128, NKT * 128], BF16, tag="wv_sb")
        cosF = res.tile([128, T], BF16, tag="cosF")
        sinF = res.tile([128, T], BF16, tag="sinF")

        xpool = ctx.enter_context(tc.tile_pool(name="xpool", bufs=3))
        qlp = ctx.enter_context(tc.tile_pool(name="qlp", bufs=2))
        vtp = ctx.enter_context(tc.tile_pool(name="vtp", bufs=2))
        swp = ctx.enter_context(tc.tile_pool(name="swp", bufs=2))
        tmpp = ctx.enter_context(tc.tile_pool(name="tmpp", bufs=2))
        ppool = ctx.enter_context(tc.tile_pool(name="ppool", bufs=3))
        ynp = ctx.enter_context(tc.tile_pool(name="ynp", bufs=2))
        recp = ctx.enter_context(tc.tile_pool(name="recp", bufs=2))
        obp = ctx.enter_context(tc.tile_pool(name="obp", bufs=2))

        pjps = ctx.enter_context(tc.tile_pool(name="pjps", bufs=2, space="PSUM"))
        spool = ctx.enter_context(tc.tile_pool(name="spool", bufs=2, space="PSUM"))
        ypool = ctx.enter_context(tc.tile_pool(name="ypool", bufs=1, space="PSUM"))
        dpool = ctx.enter_context(tc.tile_pool(name="dpool", bufs=1, space="PSUM"))
        opool = ctx.enter_context(tc.tile_pool(name="opool", bufs=2, space="PSUM"))

        # ---- deferred output-projection "filler" steps ------------------
        # Each step emits the 2-matmul PSUM pair for one (qs, n) output tile
        # plus its PSUM->SBUF copy and (once a row is complete) the DMA.
        wo_state = {"steps": [], "idx": 0}

        def make_wo_steps(ynorm, qs0, last=False):
            steps = []
            obs = {}

            def step(qs, n):
                def run():
                    if n == 0:
                        obs[qs] = obp.tile([128, 2048], BF16, tag="ob", name="ob")
                    o_ps = opool.tile([128, 512], F32, tag="o", name="o_ps")
                    nc.tensor.matmul(o_ps[:], ynorm[0][:, qs * 128:(qs + 1) * 128],
                                     wo_sb[:, n * 512:n * 512 + 512],
                                     start=True, stop=False)
                    nc.tensor.matmul(o_ps[:], ynorm[1][:, qs * 128:(qs + 1) * 128],
                                     wo_sb[:, C + n * 512:C + n * 512 + 512],
                                     start=False, stop=True)
                    ob = obs[qs]
                    if (qs * 4 + n) % 2 == 0:
                        nc.scalar.copy(ob[:, n * 512:(n + 1) * 512], o_ps[:])
                    else:
                        nc.vector.tensor_copy(ob[:, n * 512:(n + 1) * 512], o_ps[:])
                    if n == 3:
                        rows = slice(qs0 + qs * 128, qs0 + (qs + 1) * 128)
                        nc.sync.dma_start(out[rows, :], ob[:])
                return run

            for qs in range(4):
                for n in range(4):
                    steps.append(step(qs, n))
            return steps

        def fill(n):
            st = wo_state
            while n > 0 and st["idx"] < len(st["steps"]):
                st["steps"][st["idx"]]()
                st["idx"] += 1
                n -= 1

        def fill_all():
            fill(len(wo_state["steps"]))

        for it in range(NQT):
            nt = it
            b0 = 4 * it
            ts = slice(nt * 512, (nt + 1) * 512)
            qs0 = it * QTW

            # ---- x / weight DMAs (consolidated; k-weights first so the
            # first projection pass can start as soon as x chunk 0 lands) ----
            if it == 0:
                nc.sync.dma_start(
                    wk_sb[:].rearrange("p (a d) -> p a d", a=NKT),
                    wk[:, :].rearrange("(a p) d -> p a d", p=128))
                nc.gpsimd.dma_start(ident[:], identd[:])
                nc.gpsimd.dma_start(mT[:], maskt[:])
                nc.gpsimd.dma_start(ones[:], onesd[:])
                nc.gpsimd.dma_start(swpm[:], swapd[:])
            xbig = xpool.tile([128, NKT * 512], BF16, tag="xtile")
            for q4 in range(4):
                nc.sync.dma_start(
                    xbig[:, q4 * 2048:(q4 + 1) * 2048].rearrange(
                        "p (a t) -> p a t", a=4),
                    xt[q4 * 512:(q4 + 1) * 512, ts].rearrange(
                        "(a p) t -> p a t", p=128))
            xts = [xbig[:, kt * 512:(kt + 1) * 512] for kt in range(NKT)]
            if it == 0:
                nc.sync.dma_start(
                    wq_sb[:].rearrange("p (a d) -> p a d", a=NKT),
                    wq[:, :].rearrange("(a p) d -> p a d", p=128))
                nc.sync.dma_start(
                    wv_sb[:].rearrange("p (a d) -> p a d", a=NKT),
                    wv[:, :].rearrange("(a p) d -> p a d", p=128))
                nc.sync.dma_start(cosF[:], cosd[:, :])
                nc.sync.dma_start(sinF[:], sind[:, :])

            cos_t = cosF[:, ts]
            sin_t = sinF[:, ts]

            qloc = [qlp.tile([128, 512], BF16, tag=f"ql{h}", name=f"ql{h}")
                    for h in range(2)]
            # pass order k, q0, v, q1: each RoPE chain hides under the
            # following projection passes so kT/qloc are ready for attention
            wslices = [
                lambda kt: wk_sb[:, kt * 128:(kt + 1) * 128],
                lambda kt: wq_sb[:, kt * 256:kt * 256 + 128],
                lambda kt: wv_sb[:, kt * 128:(kt + 1) * 128],
                lambda kt: wq_sb[:, kt * 256 + 128:kt * 256 + 256],
            ]
            vT_t = vtp.tile([128, 512], BF16, tag="vT")
            ropedest = [kT[:, ts], qloc[0][:], None, qloc[1][:]]
            for i in range(4):
                pj = pjps.tile([128, 512], F32, tag="pj")
                for kt in range(NKT):
                    nc.tensor.matmul(pj[:], wslices[i](kt), xts[kt][:],
                                     start=(kt == 0), stop=(kt == NKT - 1))
                if i != 2:
                    # RoPE: dest = qsb*cos + swap(qsb)*sinS; the half-rotation
                    # runs on the PE (permutation matmul) to keep it off the
                    # DMA queues
                    dest = ropedest[i]
                    qsb = swp.tile([128, 512], BF16, tag="qsb")
                    nc.scalar.copy(qsb[:], pj[:])
                    sw_ps = spool.tile([128, QTW], F32, tag="s", name="sw_ps")
                    nc.tensor.matmul(sw_ps[:], swpm[:], qsb[:],
                                     start=True, stop=True)
                    ta = tmpp.tile([128, 512], BF16, tag="ta")
                    nc.vector.tensor_mul(ta[:], qsb[:], cos_t)
                    tb = tmpp.tile([128, 512], BF16, tag="tb")
                    nc.vector.tensor_mul(tb[:], sw_ps[:], sin_t)
                    nc.vector.tensor_add(dest, ta[:], tb[:])
                else:
                    nc.vector.tensor_copy(vT_t[:], pj[:])
                fill(2)

            if it == 0:
                nc.gpsimd.dma_start(mG[:], maskg[:])
                for i in range(2):
                    nc.sync.dma_start(wo_sb[:, i * C:(i + 1) * C],
                                      wo[i * 128:(i + 1) * 128, :])

            # ---- v transpose for this t-tile + incremental global K/V ----
            fill(2)
            for j in range(4):
                blk = nt * 4 + j
                tp = spool.tile([128, 512], BF16, tag="s", name="tp")
                nc.tensor.transpose(tp[:, :128], vT_t[:, j * 128:(j + 1) * 128],
                                    ident[:])
                nc.vector.tensor_copy(vN[:, blk * 128:(blk + 1) * 128], tp[:, :128])
            gsl = slice(nt * 8, (nt + 1) * 8)
            nc.vector.tensor_copy(kG[:, gsl], kT[:, ts][:, 0:512:GLOBAL_EVERY])
            nc.vector.tensor_copy(vGT[:, gsl], vT_t[:][:, 0:512:GLOBAL_EVERY])
            gw2 = 8 * (nt + 1)
            tpg = spool.tile([128, 512], BF16, tag="s", name="tpg")
            nc.tensor.transpose(tpg[:gw2, :128], vGT[:, :gw2], ident[:])
            nc.vector.tensor_copy(vG[:gw2, :], tpg[:gw2, :128])

            # ---- attention for query tile `it` (4 blocks b0..b0+3) ----
            gw = min(NG, 8 * it)   # written prefix of kG/vG; 0 for it=0
            ynorm = []
            for h in range(2):
                items = [(b0, 0, 512, None)]
                if it == 0:
                    for j in range(3):
                        items.append((j + 1, (j + 1) * 128, (3 - j) * 128, None))
                    use_glob = False
                else:
                    for j in range(4):
                        items.append((b0 - 4 + j, 0, (j + 1) * 128, j))
                    for j in range(3):
                        items.append((b0 + 1 + j, (j + 1) * 128, (3 - j) * 128, None))
                    use_glob = gw > 0

                y_ps = ypool.tile([128, QTW], F32, tag="y")
                d_ps = dpool.tile([128, QTW], F32, tag="d")
                n_items = len(items) + (1 if use_glob else 0)
                s_tiles = [None] * n_items

                def emit_qk(ii):
                    s = spool.tile([128, QTW], F32, tag="s")
                    if ii < len(items):
                        kb, qoff, w, _ = items[ii]
                        nc.tensor.matmul(
                            s[:, :w], kT[:, kb * 128:(kb + 1) * 128],
                            qloc[h][:, qoff:qoff + w],
                            start=True, stop=True)
                    else:
                        nc.tensor.matmul(s[:gw, :], kG[:, :gw], qloc[h][:],
                                         start=True, stop=True)
                    s_tiles[ii] = s

                def emit_rest(ii):
                    first = ii == 0
                    last = ii == n_items - 1
                    s = s_tiles[ii]
                    p = ppool.tile([128, QTW], BF16, tag="p")
                    if ii < len(items):
                        kb, qoff, w, tri = items[ii]
                        nc.scalar.activation(p[:, :w], s[:, :w], EXP, scale=scale)
                        if tri is not None:
                            nc.vector.tensor_mul(p[:, tri * 128:(tri + 1) * 128],
                                                 p[:, tri * 128:(tri + 1) * 128],
                                                 mT[:])
                        nc.tensor.matmul(y_ps[:, qoff:qoff + w],
                                         vN[:, kb * 128:(kb + 1) * 128], p[:, :w],
                                         start=first, stop=last)
                        nc.tensor.matmul(d_ps[:, qoff:qoff + w], ones[:, :],
                                         p[:, :w], start=first, stop=last)
                    else:
                        nc.scalar.activation(p[:gw, :], s[:gw, :], EXP, scale=scale)
                        nc.vector.tensor_mul(p[:gw, :], p[:gw, :],
                                             mG[:gw, qs0:qs0 + QTW])
                        nc.tensor.matmul(y_ps[:, :], vG[:gw, :], p[:gw, :],
                                         start=first, stop=last)
                        nc.tensor.matmul(d_ps[:, :], ones[:gw, :], p[:gw, :],
                                         start=first, stop=last)

                emit_qk(0)
                for ii in range(n_items):
                    if ii + 1 < n_items:
                        emit_qk(ii + 1)
                    fill(1)
                    emit_rest(ii)

                # d_ps holds the denominator replicated across partitions, so
                # the reciprocal is already in broadcast form for the multiply
                rbc = recp.tile([128, QTW], F32, tag="rbc")
                nc.vector.reciprocal(rbc[:], d_ps[:])
                yn = ynp.tile([128, QTW], BF16, tag=f"yn{h}", name=f"yn{h}")
                nc.vector.tensor_mul(yn[:], y_ps[:], rbc[:])
                ynorm.append(yn)
                fill(2)

            # ---- output projection: deferred as filler for the next
            # iteration's stall points ----
            fill_all()
            wo_state = {"steps": make_wo_steps(ynorm, qs0, last=(it == NQT - 1)),
                        "idx": 0}

        fill_all()

    nc.compile()
    return nc


def _host_inputs(x, w_q, w_kv_down, w_k_up, w_v_up, w_o):
    """Build the per-core input maps (host-side shard + precompute)."""
    import ml_dtypes
    BF = ml_dtypes.bfloat16
    x = np.asarray(x)
    w_q = np.asarray(w_q)
    w_kv_down = np.asarray(w_kv_down)
    w_k_up = np.asarray(w_k_up)
    w_v_up = np.asarray(w_v_up)
    w_o = np.asarray(w_o)
    x2 = np.ascontiguousarray(x.reshape(T, C).astype(np.float32))
    xt = np.ascontiguousarray(x2.T.astype(BF))

    # RoPE tables, [hd, t] layout, sign folded into sin for the swapped term
    freqs = 1.0 / (ROPE_THETA ** (np.arange(0, HD, 2, dtype=np.float64) / HD))
    emb = np.arange(T, dtype=np.float64)[:, None] * freqs[None, :]   # [T, 64]
    cos = np.concatenate([np.cos(emb), np.cos(emb)], axis=-1)        # [T, 128]
    sin = np.concatenate([np.sin(emb), np.sin(emb)], axis=-1)
    cosT = np.ascontiguousarray(cos.T.astype(BF))                    # [128, T]
    sinS = sin.T.copy()
    sinS[:64, :] *= -1.0
    sinS = np.ascontiguousarray(sinS.astype(BF))

    # fixed triangular+global mask for the b-4 key block, [k_off, q_off]
    oi = np.arange(128)
    mTm = ((oi[None, :] <= oi[:, None]) | (oi[:, None] % 64 == 0)).astype(BF)

    # global-column mask [g, q]: visible iff 64 g < 128 (q//128 - 4)
    g = np.arange(NG)
    qb = np.arange(T) // BLOCK
    mGm = (64 * g[:, None] < 128 * (qb[None, :] - 4)).astype(BF)

    onesm = np.ones((128, 128), BF)
    ident = np.eye(128, dtype=BF)
    # swap matrix: out[m] = in[(m+64)%128]  (matmul form: swapm[k,m]=1 iff
    # k == (m+64)%128)
    km = np.arange(128)
    swapm = (km[:, None] == (km[None, :] + 64) % 128).astype(BF)

    wk_f = (w_kv_down.astype(np.float32) @ w_k_up.astype(np.float32))  # [C, KVH*HD]
    wv_f = (w_kv_down.astype(np.float32) @ w_v_up.astype(np.float32))

    in_maps = []
    for c in range(N_CORES):
        h0 = 2 * c
        kv = h0 // (H // KVH)
        wq_c = np.ascontiguousarray(
            w_q[:, h0 * HD:(h0 + 2) * HD].astype(BF))
        wk_c = np.ascontiguousarray(
            wk_f[:, kv * HD:(kv + 1) * HD].astype(BF))
        wv_c = np.ascontiguousarray(
            wv_f[:, kv * HD:(kv + 1) * HD].astype(BF))
        wo_c = np.ascontiguousarray(
            w_o[h0 * HD:(h0 + 2) * HD, :].astype(BF))
        in_maps.append({
            "xt": xt, "wq": wq_c, "wk": wk_c, "wv": wv_c, "wo": wo_c,
            "cosd": cosT, "sind": sinS, "maskt": mTm, "maskg": mGm,
            "onesd": onesm, "identd": ident, "swapd": swapm,
        })
    return in_maps


def _get_module():
    if "nc" not in _CACHE:
        _CACHE["nc"] = _build_module()
    return _CACHE["nc"]


def kernel(x, w_q, w_kv_down, w_k_up, w_v_up, w_o):
    from concourse.bass_utils import run_bass_kernel_spmd

    nc = _get_module()
    in_maps = _host_inputs(x, w_q, w_kv_down, w_k_up, w_v_up, w_o)
    res = run_bass_kernel_spmd(nc, in_maps, list(range(N_CORES)))
    acc = np.zeros((T, C), np.float32)
    for c in range(N_CORES):
        acc += np.asarray(res.results[c]["out"], dtype=np.float32)
    return acc.reshape(1, T, C)


# revision 19
# speedup vs baseline: 1.2906x; 1.0685x over previous
"""Trainium2 Bass kernel for block-causal sparse attention (MLA-style KV).

Sharding: tensor-parallel over heads. 16 heads / 8 cores = 2 heads per core,
one KV head per core-pair. Each core computes q/k/v projections from the full
(transposed) x, RoPE, sparse attention for its 2 heads, and a partial output
projection; the host sums the 8 partial outputs.

Sparsity structure (T=4096, BLOCK=128, WINDOW=512, GLOBAL_EVERY=64):
for query block b, visible keys are blocks b-4..b (block b-4 masked by a fixed
triangular+global pattern) plus "global" columns j%64==0 with j < 128*(b-4).

All matmul operands are bf16 (fp32 PSUM accumulation). Scores are computed
transposed ([k, q] layout) so probabilities feed the PV and output-projection
matmuls with no transposes. Softmax denominators are accumulated with
ones-matrix matmuls directly in broadcast form ([128, q] in PSUM), so the
reciprocal feeds the normalize multiply without a partition broadcast.

The PE stream is kept dense by interleaving the previous tile's output
projection ("wo filler" pairs) into the stall points of the current tile's
pipeline: after each projection pass (while RoPE chains run on ACT/DVE) and
between attention items (while the exp chain runs on ACT). DMAs are
consolidated (x in 4 chunks/tile, single-shot weights, full-T rope tables,
row-batched output) to keep HWDGE holds off the critical path.
"""

import numpy as np

N_CORES = 8
T = 4096
C = 2048
L = 512
H = 16
KVH = 4
HD = 128
BLOCK = 128
WINDOW = 512
GLOBAL_EVERY = 64
ROPE_THETA = 10000.0

QTW = 512            # query tile width (4 blocks)
NQT = T // QTW       # 8
NKT = C // 128       # 16 contraction tiles for projections
NG = T // GLOBAL_EVERY  # 64 global columns

_CACHE = {}


def _build_module():
    import concourse.bacc as bacc
    import concourse.mybir as mybir
    import concourse.tile as tile
    from contextlib import ExitStack

    F32 = mybir.dt.float32
    BF16 = mybir.dt.bfloat16
    FP8 = mybir.dt.float8e4
    DR = mybir.MatmulPerfMode.DoubleRow
    EXP = mybir.ActivationFunctionType.Exp

    nc = bacc.Bacc("TRN2", target_bir_lowering=False, debug=False,
                   num_devices=N_CORES)

    # x and the projection weights ship as fp8 hi/lo pairs (weights
    # pre-scaled by WS=64 on the host); projections run as 3-term DoubleRow
    # fp8 matmuls (hi*hi + hi*lo + lo*hi), which the PE executes at 2x rate
    # over a 256-deep contraction.
    xh = nc.dram_tensor("xh", [C, T], FP8, kind="ExternalInput")
    xl = nc.dram_tensor("xl", [C, T], FP8, kind="ExternalInput")
    wnames = ["wk", "wq0", "wv", "wq1"]
    wdram = {}
    for wn in wnames:
        wdram[wn + "h"] = nc.dram_tensor(wn + "h", [C, HD], FP8,
                                         kind="ExternalInput")
        wdram[wn + "l"] = nc.dram_tensor(wn + "l", [C, HD], FP8,
                                         kind="ExternalInput")
    wo = nc.dram_tensor("wo", [2 * HD, C], BF16, kind="ExternalInput")
    cosd = nc.dram_tensor("cosd", [HD, T], BF16, kind="ExternalInput")
    sind = nc.dram_tensor("sind", [HD, T], BF16, kind="ExternalInput")  # sign-folded
    maskt = nc.dram_tensor("maskt", [128, 128], BF16, kind="ExternalInput")
    maskg = nc.dram_tensor("maskg", [NG, T], BF16, kind="ExternalInput")
    onesd = nc.dram_tensor("onesd", [128, 128], BF16, kind="ExternalInput")
    identd = nc.dram_tensor("identd", [128, 128], BF16, kind="ExternalInput")
    swapd = nc.dram_tensor("swapd", [128, 128], BF16, kind="ExternalInput")
    out = nc.dram_tensor("out", [T, C], BF16, kind="ExternalOutput")

    WS = 64.0
    scale = 1.0 / np.sqrt(HD) / (WS * WS)

    with tile.TileContext(nc) as tc, ExitStack() as ctx:
        res = ctx.enter_context(tc.tile_pool(name="res", bufs=1))
        kT = res.tile([128, T], BF16, tag="kT")
        vN = res.tile([128, T], BF16, tag="vN")
        kG = res.tile([128, NG], BF16, tag="kG")
        vG = res.tile([64, 128], BF16, tag="vG")
        vGT = res.tile([128, NG], BF16, tag="vGT")
        mT = res.tile([128, 128], BF16, tag="mT")
        mG = res.tile([NG, T], BF16, tag="mG")
        ones = res.tile([128, 128], BF16, tag="ones")
        ident = res.tile([128, 128], BF16, tag="ident")
        swpm = res.tile([128, 128], BF16, tag="swpm")
        wo_sb = res.tile([128, 2 * C], BF16, tag="wo_sb")
        wsb = {}
        for wn in wnames:
            for sf in ("h", "l"):
                wsb[wn + sf] = res.tile([128, NKT * HD], FP8, tag=wn + sf,
                                        name=wn + sf)
        cosF = res.tile([128, T], BF16, tag="cosF")
        sinF = res.tile([128, T], BF16, tag="sinF")

        xpool = ctx.enter_context(tc.tile_pool(name="xpool", bufs=3))
        qlp = ctx.enter_context(tc.tile_pool(name="qlp", bufs=2))
        vtp = ctx.enter_context(tc.tile_pool(name="vtp", bufs=2))
        swp = ctx.enter_context(tc.tile_pool(name="swp", bufs=2))
        tmpp = ctx.enter_context(tc.tile_pool(name="tmpp", bufs=2))
        ppool = ctx.enter_context(tc.tile_pool(name="ppool", bufs=3))
        ynp = ctx.enter_context(tc.tile_pool(name="ynp", bufs=2))
        recp = ctx.enter_context(tc.tile_pool(name="recp", bufs=2))
        obp = ctx.enter_context(tc.tile_pool(name="obp", bufs=2))

        pjps = ctx.enter_context(tc.tile_pool(name="pjps", bufs=2, space="PSUM"))
        spool = ctx.enter_context(tc.tile_pool(name="spool", bufs=2, space="PSUM"))
        ypool = ctx.enter_context(tc.tile_pool(name="ypool", bufs=1, space="PSUM"))
        dpool = ctx.enter_context(tc.tile_pool(name="dpool", bufs=1, space="PSUM"))
        opool = ctx.enter_context(tc.tile_pool(name="opool", bufs=2, space="PSUM"))

        # ---- deferred output-projection "filler" steps ------------------
        # Each step emits the 2-matmul PSUM pair for one (qs, n) output tile
        # plus its PSUM->SBUF copy and (once a row is complete) the DMA.
        wo_state = {"steps": [], "idx": 0}

        def make_wo_steps(ynorm, qs0, last=False):
            steps = []
            obs = {}

            def step(qs, n):
                def run():
                    if n == 0:
                        obs[qs] = obp.tile([128, 2048], BF16, tag="ob", name="ob")
                    o_ps = opool.tile([128, 512], F32, tag="o", name="o_ps")
                    nc.tensor.matmul(o_ps[:], ynorm[0][:, qs * 128:(qs + 1) * 128],
                                     wo_sb[:, n * 512:n * 512 + 512],
                                     start=True, stop=False)
                    nc.tensor.matmul(o_ps[:], ynorm[1][:, qs * 128:(qs + 1) * 128],
                                     wo_sb[:, C + n * 512:C + n * 512 + 512],
                                     start=False, stop=True)
                    ob = obs[qs]
                    if (qs * 4 + n) % 2 == 0:
                        nc.scalar.mul(ob[:, n * 512:(n + 1) * 512], o_ps[:],
                                      1.0 / WS)
                    else:
                        nc.vector.tensor_scalar_mul(
                            ob[:, n * 512:(n + 1) * 512], o_ps[:], 1.0 / WS)
                    if n == 3:
                        rows = slice(qs0 + qs * 128, qs0 + (qs + 1) * 128)
                        nc.sync.dma_start(out[rows, :], ob[:])
                return run

            for qs in range(4):
                for n in range(4):
                    steps.append(step(qs, n))
            return steps

        def fill(n):
            st = wo_state
            while n > 0 and st["idx"] < len(st["steps"]):
                st["steps"][st["idx"]]()
                st["idx"] += 1
                n -= 1

        def fill_all():
            fill(len(wo_state["steps"]))

        for it in range(NQT):
            nt = it
            b0 = 4 * it
            ts = slice(nt * 512, (nt + 1) * 512)
            qs0 = it * QTW

            # ---- x / weight DMAs (consolidated; k-weights first so the
            # first projection pass can start as soon as x chunk 0 lands) ----
            if it == 0:
                for sf in ("h", "l"):
                    nc.sync.dma_start(
                        wsb["wk" + sf][:].rearrange("p (a d) -> p a d", a=NKT),
                        wdram["wk" + sf][:, :].rearrange("(a p) d -> p a d",
                                                         p=128))
                nc.gpsimd.dma_start(ident[:], identd[:])
                nc.gpsimd.dma_start(mT[:], maskt[:])
                nc.gpsimd.dma_start(ones[:], onesd[:])
                nc.gpsimd.dma_start(swpm[:], swapd[:])
            xh_sb = xpool.tile([128, NKT * 512], FP8, tag="xh")
            xl_sb = xpool.tile([128, NKT * 512], FP8, tag="xl")
            for q4 in range(4):
                for xsb, xdr in ((xh_sb, xh), (xl_sb, xl)):
                    nc.sync.dma_start(
                        xsb[:, q4 * 2048:(q4 + 1) * 2048].rearrange(
                            "p (a t) -> p a t", a=4),
                        xdr[q4 * 512:(q4 + 1) * 512, ts].rearrange(
                            "(a p) t -> p a t", p=128))
            if it == 0:
                for wn in ("wq0", "wv", "wq1"):
                    for sf in ("h", "l"):
                        nc.sync.dma_start(
                            wsb[wn + sf][:].rearrange("p (a d) -> p a d",
                                                      a=NKT),
                            wdram[wn + sf][:, :].rearrange("(a p) d -> p a d",
                                                           p=128))
                nc.sync.dma_start(cosF[:], cosd[:, :])
                nc.sync.dma_start(sinF[:], sind[:, :])

            cos_t = cosF[:, ts]
            sin_t = sinF[:, ts]

            qloc = [qlp.tile([128, 512], BF16, tag=f"ql{h}", name=f"ql{h}")
                    for h in range(2)]
            # pass order k, q0, v, q1: each RoPE chain hides under the
            # following projection passes so kT/qloc are ready for attention
            vT_t = vtp.tile([128, 512], BF16, tag="vT")
            ropedest = [kT[:, ts], qloc[0][:], None, qloc[1][:]]
            for i in range(4):
                wn = wnames[i]
                pj = pjps.tile([128, 512], F32, tag="pj")
                for cp in range(NKT // 2):
                    wh = wsb[wn + "h"][:, 2 * cp * 128:(2 * cp + 2) * 128]\
                        .rearrange("p (a m) -> p a m", a=2)
                    wl = wsb[wn + "l"][:, 2 * cp * 128:(2 * cp + 2) * 128]\
                        .rearrange("p (a m) -> p a m", a=2)
                    xhp = xh_sb[:, 2 * cp * 512:(2 * cp + 2) * 512]\
                        .rearrange("p (a t) -> p a t", a=2)
                    xlp = xl_sb[:, 2 * cp * 512:(2 * cp + 2) * 512]\
                        .rearrange("p (a t) -> p a t", a=2)
                    nc.tensor.matmul(pj[:], wh, xhp, perf_mode=DR,
                                     start=(cp == 0), stop=False)
                    nc.tensor.matmul(pj[:], wl, xhp, perf_mode=DR,
                                     start=False, stop=False)
                    nc.tensor.matmul(pj[:], wh, xlp, perf_mode=DR,
                                     start=False, stop=(cp == NKT // 2 - 1))
                if i != 2:
                    # RoPE: dest = qsb*cos + swap(qsb)*sinS; the half-rotation
                    # runs on the PE (permutation matmul) to keep it off the
                    # DMA queues
                    dest = ropedest[i]
                    qsb = swp.tile([128, 512], BF16, tag="qsb")
                    nc.scalar.copy(qsb[:], pj[:])
                    sw_ps = spool.tile([128, QTW], F32, tag="s", name="sw_ps")
                    nc.tensor.matmul(sw_ps[:], swpm[:], qsb[:],
                                     start=True, stop=True)
                    ta = tmpp.tile([128, 512], BF16, tag="ta")
                    nc.vector.tensor_mul(ta[:], qsb[:], cos_t)
                    tb = tmpp.tile([128, 512], BF16, tag="tb")
                    nc.vector.tensor_mul(tb[:], sw_ps[:], sin_t)
                    nc.vector.tensor_add(dest, ta[:], tb[:])
                else:
                    nc.vector.tensor_copy(vT_t[:], pj[:])
                fill(2)

            if it == 0:
                nc.gpsimd.dma_start(mG[:], maskg[:])
                for i in range(2):
                    nc.sync.dma_start(wo_sb[:, i * C:(i + 1) * C],
                                      wo[i * 128:(i + 1) * 128, :])

            # ---- v transpose for this t-tile + incremental global K/V ----
            fill(2)
            for j in range(4):
                blk = nt * 4 + j
                tp = spool.tile([128, 512], BF16, tag="s", name="tp")
                nc.tensor.transpose(tp[:, :128], vT_t[:, j * 128:(j + 1) * 128],
                                    ident[:])
                nc.vector.tensor_copy(vN[:, blk * 128:(blk + 1) * 128], tp[:, :128])
            gsl = slice(nt * 8, (nt + 1) * 8)
            nc.vector.tensor_copy(kG[:, gsl], kT[:, ts][:, 0:512:GLOBAL_EVERY])
            nc.vector.tensor_copy(vGT[:, gsl], vT_t[:][:, 0:512:GLOBAL_EVERY])
            gw2 = 8 * (nt + 1)
            tpg = spool.tile([128, 512], BF16, tag="s", name="tpg")
            nc.tensor.transpose(tpg[:gw2, :128], vGT[:, :gw2], ident[:])
            nc.vector.tensor_copy(vG[:gw2, :], tpg[:gw2, :128])

            # ---- attention for query tile `it` (4 blocks b0..b0+3) ----
            gw = min(NG, 8 * it)   # written prefix of kG/vG; 0 for it=0
            ynorm = []
            for h in range(2):
                items = [(b0, 0, 512, None)]
                if it == 0:
                    for j in range(3):
                        items.append((j + 1, (j + 1) * 128, (3 - j) * 128, None))
                    use_glob = False
                else:
                    for j in range(4):
                        items.append((b0 - 4 + j, 0, (j + 1) * 128, j))
                    for j in range(3):
                        items.append((b0 + 1 + j, (j + 1) * 128, (3 - j) * 128, None))
                    use_glob = gw > 0

                y_ps = ypool.tile([128, QTW], F32, tag="y")
                d_ps = dpool.tile([128, QTW], F32, tag="d")
                n_items = len(items) + (1 if use_glob else 0)
                s_tiles = [None] * n_items

                def emit_qk(ii):
                    s = spool.tile([128, QTW], F32, tag="s")
                    if ii < len(items):
                        kb, qoff, w, _ = items[ii]
                        nc.tensor.matmul(
                            s[:, :w], kT[:, kb * 128:(kb + 1) * 128],
                            qloc[h][:, qoff:qoff + w],
                            start=True, stop=True)
                    else:
                        nc.tensor.matmul(s[:gw, :], kG[:, :gw], qloc[h][:],
                                         start=True, stop=True)
                    s_tiles[ii] = s

                def emit_rest(ii):
                    first = ii == 0
                    last = ii == n_items - 1
                    s = s_tiles[ii]
                    p = ppool.tile([128, QTW], BF16, tag="p")
                    if ii < len(items):
                        kb, qoff, w, tri = items[ii]
                        nc.scalar.activation(p[:, :w], s[:, :w], EXP, scale=scale)
                        if tri is not None:
                            nc.vector.tensor_mul(p[:, tri * 128:(tri + 1) * 128],
                                                 p[:, tri * 128:(tri + 1) * 128],
                                                 mT[:])
                        nc.tensor.matmul(y_ps[:, qoff:qoff + w],
                                         vN[:, kb * 128:(kb + 1) * 128], p[:, :w],
                                         start=first, stop=last)
                        nc.tensor.matmul(d_ps[:, qoff:qoff + w], ones[:, :],
                                         p[:, :w], start=first, stop=last)
                    else:
                        nc.scalar.activation(p[:gw, :], s[:gw, :], EXP, scale=scale)
                        nc.vector.tensor_mul(p[:gw, :], p[:gw, :],
                                             mG[:gw, qs0:qs0 + QTW])
                        nc.tensor.matmul(y_ps[:, :], vG[:gw, :], p[:gw, :],
                                         start=first, stop=last)
                        nc.tensor.matmul(d_ps[:, :], ones[:gw, :], p[:gw, :],
                                         start=first, stop=last)

                emit_qk(0)
                for ii in range(n_items):
                    if ii + 1 < n_items:
                        emit_qk(ii + 1)
                    fill(1)
                    emit_rest(ii)

                # d_ps holds the denominator replicated across partitions, so
                # the reciprocal is already in broadcast form for the multiply
                rbc = recp.tile([128, QTW], F32, tag="rbc")
                nc.vector.reciprocal(rbc[:], d_ps[:])
                yn = ynp.tile([128, QTW], BF16, tag=f"yn{h}", name=f"yn{h}")
                nc.vector.tensor_mul(yn[:], y_ps[:], rbc[:])
                ynorm.append(yn)
                fill(2)

            # ---- output projection: deferred as filler for the next
            # iteration's stall points ----
            fill_all()
            wo_state = {"steps": make_wo_steps(ynorm, qs0, last=(it == NQT - 1)),
                        "idx": 0}

        fill_all()

    nc.compile()
    return nc


def _host_inputs(x, w_q, w_kv_down, w_k_up, w_v_up, w_o):
    """Build the per-core input maps (host-side shard + precompute)."""
    import ml_dtypes
    BF = ml_dtypes.bfloat16
    E4 = (ml_dtypes.float8_e4m3fn if hasattr(ml_dtypes, "float8_e4m3fn")
          else ml_dtypes.float8_e4m3)
    WS = 64.0
    x = np.asarray(x)
    w_q = np.asarray(w_q)
    w_kv_down = np.asarray(w_kv_down)
    w_k_up = np.asarray(w_k_up)
    w_v_up = np.asarray(w_v_up)
    w_o = np.asarray(w_o)
    x2 = np.ascontiguousarray(x.reshape(T, C).astype(np.float32))
    xt = np.ascontiguousarray(x2.T)

    def hilo(a):
        hi = a.astype(E4)
        lo = (a - hi.astype(np.float32)).astype(E4)
        return np.ascontiguousarray(hi), np.ascontiguousarray(lo)

    xt_h, xt_l = hilo(xt)

    # RoPE tables, [hd, t] layout, sign folded into sin for the swapped term
    freqs = 1.0 / (ROPE_THETA ** (np.arange(0, HD, 2, dtype=np.float64) / HD))
    emb = np.arange(T, dtype=np.float64)[:, None] * freqs[None, :]   # [T, 64]
    cos = np.concatenate([np.cos(emb), np.cos(emb)], axis=-1)        # [T, 128]
    sin = np.concatenate([np.sin(emb), np.sin(emb)], axis=-1)
    cosT = np.ascontiguousarray(cos.T.astype(BF))                    # [128, T]
    sinS = sin.T.copy()
    sinS[:64, :] *= -1.0
    sinS = np.ascontiguousarray(sinS.astype(BF))

    # fixed triangular+global mask for the b-4 key block, [k_off, q_off]
    oi = np.arange(128)
    mTm = ((oi[None, :] <= oi[:, None]) | (oi[:, None] % 64 == 0)).astype(BF)

    # global-column mask [g, q]: visible iff 64 g < 128 (q//128 - 4)
    g = np.arange(NG)
    qb = np.arange(T) // BLOCK
    mGm = (64 * g[:, None] < 128 * (qb[None, :] - 4)).astype(BF)

    onesm = np.ones((128, 128), BF)
    ident = np.eye(128, dtype=BF)
    # swap matrix: out[m] = in[(m+64)%128]  (matmul form: swapm[k,m]=1 iff
    # k == (m+64)%128)
    km = np.arange(128)
    swapm = (km[:, None] == (km[None, :] + 64) % 128).astype(BF)

    wk_f = (w_kv_down.astype(np.float32) @ w_k_up.astype(np.float32))  # [C, KVH*HD]
    wv_f = (w_kv_down.astype(np.float32) @ w_v_up.astype(np.float32))

    in_maps = []
    for c in range(N_CORES):
        h0 = 2 * c
        kv = h0 // (H // KVH)
        wq0_h, wq0_l = hilo(w_q[:, h0 * HD:(h0 + 1) * HD].astype(np.float32) * WS)
        wq1_h, wq1_l = hilo(w_q[:, (h0 + 1) * HD:(h0 + 2) * HD].astype(np.float32) * WS)
        wk_h, wk_l = hilo(wk_f[:, kv * HD:(kv + 1) * HD] * WS)
        wv_h, wv_l = hilo(wv_f[:, kv * HD:(kv + 1) * HD] * WS)
        wo_c = np.ascontiguousarray(
            w_o[h0 * HD:(h0 + 2) * HD, :].astype(BF))
        in_maps.append({
            "xh": xt_h, "xl": xt_l,
            "wq0h": wq0_h, "wq0l": wq0_l, "wq1h": wq1_h, "wq1l": wq1_l,
            "wkh": wk_h, "wkl": wk_l, "wvh": wv_h, "wvl": wv_l,
            "wo": wo_c,
            "cosd": cosT, "sind": sinS, "maskt": mTm, "maskg": mGm,
            "onesd": onesm, "identd": ident, "swapd": swapm,
        })
    return in_maps


def _get_module():
    if "nc" not in _CACHE:
        _CACHE["nc"] = _build_module()
    return _CACHE["nc"]


def kernel(x, w_q, w_kv_down, w_k_up, w_v_up, w_o):
    from concourse.bass_utils import run_bass_kernel_spmd

    nc = _get_module()
    in_maps = _host_inputs(x, w_q, w_kv_down, w_k_up, w_v_up, w_o)
    res = run_bass_kernel_spmd(nc, in_maps, list(range(N_CORES)))
    acc = np.zeros((T, C), np.float32)
    for c in range(N_CORES):
        acc += np.asarray(res.results[c]["out"], dtype=np.float32)
    return acc.reshape(1, T, C)


# revision 21
# speedup vs baseline: 1.3235x; 1.0255x over previous
"""Trainium2 Bass kernel for block-causal sparse attention (MLA-style KV).

Sharding: tensor-parallel over heads. 16 heads / 8 cores = 2 heads per core,
one KV head per core-pair. Each core computes q/k/v projections from the full
(transposed) x, RoPE, sparse attention for its 2 heads, and a partial output
projection; the host sums the 8 partial outputs.

Sparsity structure (T=4096, BLOCK=128, WINDOW=512, GLOBAL_EVERY=64):
for query block b, visible keys are blocks b-4..b (block b-4 masked by a fixed
triangular+global pattern) plus "global" columns j%64==0 with j < 128*(b-4).

All matmul operands are bf16 (fp32 PSUM accumulation). Scores are computed
transposed ([k, q] layout) so probabilities feed the PV and output-projection
matmuls with no transposes. Softmax denominators are accumulated with
ones-matrix matmuls directly in broadcast form ([128, q] in PSUM), so the
reciprocal feeds the normalize multiply without a partition broadcast.

The PE stream is kept dense by interleaving the previous tile's output
projection ("wo filler" pairs) into the stall points of the current tile's
pipeline: after each projection pass (while RoPE chains run on ACT/DVE) and
between attention items (while the exp chain runs on ACT). DMAs are
consolidated (x in 4 chunks/tile, single-shot weights, full-T rope tables,
row-batched output) to keep HWDGE holds off the critical path.
"""

import numpy as np

N_CORES = 8
T = 4096
C = 2048
L = 512
H = 16
KVH = 4
HD = 128
BLOCK = 128
WINDOW = 512
GLOBAL_EVERY = 64
ROPE_THETA = 10000.0

QTW = 512            # query tile width (4 blocks)
NQT = T // QTW       # 8
NKT = C // 128       # 16 contraction tiles for projections
NG = T // GLOBAL_EVERY  # 64 global columns

_CACHE = {}


def _build_module():
    import concourse.bacc as bacc
    import concourse.mybir as mybir
    import concourse.tile as tile
    from contextlib import ExitStack

    F32 = mybir.dt.float32
    BF16 = mybir.dt.bfloat16
    FP8 = mybir.dt.float8e4
    DR = mybir.MatmulPerfMode.DoubleRow
    EXP = mybir.ActivationFunctionType.Exp

    nc = bacc.Bacc("TRN2", target_bir_lowering=False, debug=False,
                   num_devices=N_CORES)

    # x and the projection weights ship as fp8 hi/lo pairs (weights
    # pre-scaled by WS=64 on the host); projections run as 3-term DoubleRow
    # fp8 matmuls (hi*hi + hi*lo + lo*hi), which the PE executes at 2x rate
    # over a 256-deep contraction.
    xh = nc.dram_tensor("xh", [C, T], FP8, kind="ExternalInput")
    xl = nc.dram_tensor("xl", [C, T], FP8, kind="ExternalInput")
    wnames = ["wk", "wq0", "wv", "wq1"]
    wdram = {}
    for wn in wnames:
        wdram[wn + "h"] = nc.dram_tensor(wn + "h", [C, HD], FP8,
                                         kind="ExternalInput")
        wdram[wn + "l"] = nc.dram_tensor(wn + "l", [C, HD], FP8,
                                         kind="ExternalInput")
    wo = nc.dram_tensor("wo", [2 * HD, C], BF16, kind="ExternalInput")
    cosd = nc.dram_tensor("cosd", [HD, T], BF16, kind="ExternalInput")
    sind = nc.dram_tensor("sind", [HD, T], BF16, kind="ExternalInput")  # sign-folded
    maskt = nc.dram_tensor("maskt", [128, 128], BF16, kind="ExternalInput")
    maskg = nc.dram_tensor("maskg", [NG, T], BF16, kind="ExternalInput")
    onesd = nc.dram_tensor("onesd", [128, 128], BF16, kind="ExternalInput")
    identd = nc.dram_tensor("identd", [128, 128], BF16, kind="ExternalInput")
    swapd = nc.dram_tensor("swapd", [128, 128], BF16, kind="ExternalInput")
    out = nc.dram_tensor("out", [T, C], BF16, kind="ExternalOutput")

    WS = 64.0
    scale = 1.0 / np.sqrt(HD) / (WS * WS)

    with tile.TileContext(nc) as tc, ExitStack() as ctx:
        res = ctx.enter_context(tc.tile_pool(name="res", bufs=1))
        kT = res.tile([128, T], BF16, tag="kT")
        vN = res.tile([128, T], BF16, tag="vN")
        kG = res.tile([128, NG], BF16, tag="kG")
        vG = res.tile([64, 128], BF16, tag="vG")
        vGT = res.tile([128, NG], BF16, tag="vGT")
        mT = res.tile([128, 128], BF16, tag="mT")
        mG = res.tile([NG, T], BF16, tag="mG")
        ones = res.tile([128, 128], BF16, tag="ones")
        ident = res.tile([128, 128], BF16, tag="ident")
        swpm = res.tile([128, 128], BF16, tag="swpm")
        wo_sb = res.tile([128, 2 * C], BF16, tag="wo_sb")
        wsb = {}
        for wn in wnames:
            for sf in ("h", "l"):
                wsb[wn + sf] = res.tile([128, NKT * HD], FP8, tag=wn + sf,
                                        name=wn + sf)
        cosF = res.tile([128, T], BF16, tag="cosF")
        sinF = res.tile([128, T], BF16, tag="sinF")

        xpool = ctx.enter_context(tc.tile_pool(name="xpool", bufs=3))
        qlp = ctx.enter_context(tc.tile_pool(name="qlp", bufs=2))
        vtp = ctx.enter_context(tc.tile_pool(name="vtp", bufs=2))
        swp = ctx.enter_context(tc.tile_pool(name="swp", bufs=2))
        tmpp = ctx.enter_context(tc.tile_pool(name="tmpp", bufs=2))
        ppool = ctx.enter_context(tc.tile_pool(name="ppool", bufs=3))
        ynp = ctx.enter_context(tc.tile_pool(name="ynp", bufs=2))
        recp = ctx.enter_context(tc.tile_pool(name="recp", bufs=2))
        obp = ctx.enter_context(tc.tile_pool(name="obp", bufs=2))

        pjps = ctx.enter_context(tc.tile_pool(name="pjps", bufs=2, space="PSUM"))
        spool = ctx.enter_context(tc.tile_pool(name="spool", bufs=2, space="PSUM"))
        ypool = ctx.enter_context(tc.tile_pool(name="ypool", bufs=1, space="PSUM"))
        dpool = ctx.enter_context(tc.tile_pool(name="dpool", bufs=1, space="PSUM"))
        opool = ctx.enter_context(tc.tile_pool(name="opool", bufs=2, space="PSUM"))

        # ---- deferred output-projection "filler" steps ------------------
        # Each step emits the 2-matmul PSUM pair for one (qs, n) output tile
        # plus its PSUM->SBUF copy and (once a row is complete) the DMA.
        wo_state = {"steps": [], "idx": 0}

        def make_wo_steps(ynorm, qs0, last=False):
            steps = []
            obs = {}

            def step(qs, n):
                def run():
                    if n == 0:
                        obs[qs] = obp.tile([128, 2048], BF16, tag="ob", name="ob")
                    o_ps = opool.tile([128, 512], F32, tag="o", name="o_ps")
                    nc.tensor.matmul(o_ps[:], ynorm[0][:, qs * 128:(qs + 1) * 128],
                                     wo_sb[:, n * 512:n * 512 + 512],
                                     start=True, stop=False)
                    nc.tensor.matmul(o_ps[:], ynorm[1][:, qs * 128:(qs + 1) * 128],
                                     wo_sb[:, C + n * 512:C + n * 512 + 512],
                                     start=False, stop=True)
                    ob = obs[qs]
                    if (qs * 4 + n) % 2 == 0:
                        nc.scalar.mul(ob[:, n * 512:(n + 1) * 512], o_ps[:],
                                      1.0 / WS)
                    else:
                        nc.vector.tensor_scalar_mul(
                            ob[:, n * 512:(n + 1) * 512], o_ps[:], 1.0 / WS)
                    if n == 3:
                        rows = slice(qs0 + qs * 128, qs0 + (qs + 1) * 128)
                        nc.sync.dma_start(out[rows, :], ob[:])
                return run

            for qs in range(4):
                for n in range(4):
                    steps.append(step(qs, n))
            return steps

        def fill(n):
            st = wo_state
            while n > 0 and st["idx"] < len(st["steps"]):
                st["steps"][st["idx"]]()
                st["idx"] += 1
                n -= 1

        def fill_all():
            fill(len(wo_state["steps"]))

        for it in range(NQT):
            nt = it
            b0 = 4 * it
            ts = slice(nt * 512, (nt + 1) * 512)
            qs0 = it * QTW

            # ---- x / weight DMAs (consolidated; k-weights first so the
            # first projection pass can start as soon as x chunk 0 lands) ----
            if it == 0:
                for sf in ("h", "l"):
                    nc.sync.dma_start(
                        wsb["wk" + sf][:].rearrange("p (a d) -> p a d", a=NKT),
                        wdram["wk" + sf][:, :].rearrange("(a p) d -> p a d",
                                                         p=128))
                nc.gpsimd.dma_start(ident[:], identd[:])
                nc.gpsimd.dma_start(mT[:], maskt[:])
                nc.gpsimd.dma_start(ones[:], onesd[:])
                nc.gpsimd.dma_start(swpm[:], swapd[:])
            xh_sb = xpool.tile([128, NKT * 512], FP8, tag="xh")
            xl_sb = xpool.tile([128, NKT * 512], FP8, tag="xl")
            for q4 in range(4):
                for xsb, xdr in ((xh_sb, xh), (xl_sb, xl)):
                    nc.sync.dma_start(
                        xsb[:, q4 * 2048:(q4 + 1) * 2048].rearrange(
                            "p (a t) -> p a t", a=4),
                        xdr[q4 * 512:(q4 + 1) * 512, ts].rearrange(
                            "(a p) t -> p a t", p=128))
            if it == 0:
                for wn in ("wq0", "wv", "wq1"):
                    for sf in ("h", "l"):
                        nc.sync.dma_start(
                            wsb[wn + sf][:].rearrange("p (a d) -> p a d",
                                                      a=NKT),
                            wdram[wn + sf][:, :].rearrange("(a p) d -> p a d",
                                                           p=128))
                nc.sync.dma_start(cosF[:], cosd[:, :])
                nc.sync.dma_start(sinF[:], sind[:, :])

            cos_t = cosF[:, ts]
            sin_t = sinF[:, ts]

            qloc = [qlp.tile([128, 512], BF16, tag=f"ql{h}", name=f"ql{h}")
                    for h in range(2)]
            # pass order k, q0, v, q1: each RoPE chain hides under the
            # following projection passes so kT/qloc are ready for attention
            vT_t = vtp.tile([128, 512], BF16, tag="vT")
            ropedest = [kT[:, ts], qloc[0][:], None, qloc[1][:]]

            def emit_pass_cp(i, pj, cp):
                wn = wnames[i]
                wh = wsb[wn + "h"][:, 2 * cp * 128:(2 * cp + 2) * 128]\
                    .rearrange("p (a m) -> p a m", a=2)
                wl = wsb[wn + "l"][:, 2 * cp * 128:(2 * cp + 2) * 128]\
                    .rearrange("p (a m) -> p a m", a=2)
                xhp = xh_sb[:, 2 * cp * 512:(2 * cp + 2) * 512]\
                    .rearrange("p (a t) -> p a t", a=2)
                xlp = xl_sb[:, 2 * cp * 512:(2 * cp + 2) * 512]\
                    .rearrange("p (a t) -> p a t", a=2)
                nc.tensor.matmul(pj[:], wh, xhp, perf_mode=DR,
                                 start=(cp == 0), stop=False)
                nc.tensor.matmul(pj[:], wl, xhp, perf_mode=DR,
                                 start=False, stop=False)
                nc.tensor.matmul(pj[:], wh, xlp, perf_mode=DR,
                                 start=False, stop=(cp == NKT // 2 - 1))

            def emit_pass_tail(i, pj):
                if i != 2:
                    # RoPE: dest = qsb*cos + swap(qsb)*sinS; the half-rotation
                    # runs on the PE (permutation matmul) to keep it off the
                    # DMA queues
                    dest = ropedest[i]
                    qsb = swp.tile([128, 512], BF16, tag="qsb")
                    nc.scalar.copy(qsb[:], pj[:])
                    sw_ps = spool.tile([128, QTW], F32, tag="s", name="sw_ps")
                    nc.tensor.matmul(sw_ps[:], swpm[:], qsb[:],
                                     start=True, stop=True)
                    ta = tmpp.tile([128, 512], BF16, tag="ta")
                    nc.vector.tensor_mul(ta[:], qsb[:], cos_t)
                    tb = tmpp.tile([128, 512], BF16, tag="tb")
                    nc.vector.tensor_mul(tb[:], sw_ps[:], sin_t)
                    nc.vector.tensor_add(dest, ta[:], tb[:])
                else:
                    nc.vector.tensor_copy(vT_t[:], pj[:])

            # passes k, q0, v run eagerly; pass q1 is deferred and dripped
            # into h0's attention items as PE filler
            for i in range(3):
                pj = pjps.tile([128, 512], F32, tag="pj")
                for cp in range(NKT // 2):
                    emit_pass_cp(i, pj, cp)
                emit_pass_tail(i, pj)
                fill(2)
            pj_q1 = pjps.tile([128, 512], F32, tag="pj")
            q1_state = {"cp": 0}

            def drip_q1(n):
                while n > 0 and q1_state["cp"] < NKT // 2:
                    emit_pass_cp(3, pj_q1, q1_state["cp"])
                    q1_state["cp"] += 1
                    if q1_state["cp"] == NKT // 2:
                        emit_pass_tail(3, pj_q1)
                    n -= 1

            if it == 0:
                nc.gpsimd.dma_start(mG[:], maskg[:])
                for i in range(2):
                    nc.sync.dma_start(wo_sb[:, i * C:(i + 1) * C],
                                      wo[i * 128:(i + 1) * 128, :])

            # ---- v transpose for this t-tile + incremental global K/V ----
            fill(2)
            for j in range(4):
                blk = nt * 4 + j
                tp = spool.tile([128, 512], BF16, tag="s", name="tp")
                nc.tensor.transpose(tp[:, :128], vT_t[:, j * 128:(j + 1) * 128],
                                    ident[:])
                nc.vector.tensor_copy(vN[:, blk * 128:(blk + 1) * 128], tp[:, :128])
            gsl = slice(nt * 8, (nt + 1) * 8)
            nc.vector.tensor_copy(kG[:, gsl], kT[:, ts][:, 0:512:GLOBAL_EVERY])
            nc.vector.tensor_copy(vGT[:, gsl], vT_t[:][:, 0:512:GLOBAL_EVERY])
            gw2 = 8 * (nt + 1)
            tpg = spool.tile([128, 512], BF16, tag="s", name="tpg")
            nc.tensor.transpose(tpg[:gw2, :128], vGT[:, :gw2], ident[:])
            nc.vector.tensor_copy(vG[:gw2, :], tpg[:gw2, :128])

            # ---- attention for query tile `it` (4 blocks b0..b0+3) ----
            gw = min(NG, 8 * it)   # written prefix of kG/vG; 0 for it=0
            ynorm = []
            for h in range(2):
                items = [(b0, 0, 512, None)]
                if it == 0:
                    for j in range(3):
                        items.append((j + 1, (j + 1) * 128, (3 - j) * 128, None))
                    use_glob = False
                else:
                    for j in range(4):
                        items.append((b0 - 4 + j, 0, (j + 1) * 128, j))
                    for j in range(3):
                        items.append((b0 + 1 + j, (j + 1) * 128, (3 - j) * 128, None))
                    use_glob = gw > 0

                y_ps = ypool.tile([128, QTW], F32, tag="y")
                d_ps = dpool.tile([128, QTW], F32, tag="d")
                n_items = len(items) + (1 if use_glob else 0)
                s_tiles = [None] * n_items

                def emit_qk(ii):
                    s = spool.tile([128, QTW], F32, tag="s")
                    if ii < len(items):
                        kb, qoff, w, _ = items[ii]
                        nc.tensor.matmul(
                            s[:, :w], kT[:, kb * 128:(kb + 1) * 128],
                            qloc[h][:, qoff:qoff + w],
                            start=True, stop=True)
                    else:
                        nc.tensor.matmul(s[:gw, :], kG[:, :gw], qloc[h][:],
                                         start=True, stop=True)
                    s_tiles[ii] = s

                def emit_rest(ii):
                    first = ii == 0
                    last = ii == n_items - 1
                    s = s_tiles[ii]
                    p = ppool.tile([128, QTW], BF16, tag="p")
                    if ii < len(items):
                        kb, qoff, w, tri = items[ii]
                        nc.scalar.activation(p[:, :w], s[:, :w], EXP, scale=scale)
                        if tri is not None:
                            nc.vector.tensor_mul(p[:, tri * 128:(tri + 1) * 128],
                                                 p[:, tri * 128:(tri + 1) * 128],
                                                 mT[:])
                        nc.tensor.matmul(y_ps[:, qoff:qoff + w],
                                         vN[:, kb * 128:(kb + 1) * 128], p[:, :w],
                                         start=first, stop=last)
                        nc.tensor.matmul(d_ps[:, qoff:qoff + w], ones[:, :],
                                         p[:, :w], start=first, stop=last)
                    else:
                        nc.scalar.activation(p[:gw, :], s[:gw, :], EXP, scale=scale)
                        nc.vector.tensor_mul(p[:gw, :], p[:gw, :],
                                             mG[:gw, qs0:qs0 + QTW])
                        nc.tensor.matmul(y_ps[:, :], vG[:gw, :], p[:gw, :],
                                         start=first, stop=last)
                        nc.tensor.matmul(d_ps[:, :], ones[:gw, :], p[:gw, :],
                                         start=first, stop=last)

                emit_qk(0)
                for ii in range(n_items):
                    if ii + 1 < n_items:
                        emit_qk(ii + 1)
                    if h == 0:
                        drip_q1(1)
                    else:
                        fill(1)
                    emit_rest(ii)
                if h == 0:
                    drip_q1(NKT // 2)

                # d_ps holds the denominator replicated across partitions, so
                # the reciprocal is already in broadcast form for the multiply
                rbc = recp.tile([128, QTW], F32, tag="rbc")
                nc.vector.reciprocal(rbc[:], d_ps[:])
                yn = ynp.tile([128, QTW], BF16, tag=f"yn{h}", name=f"yn{h}")
                nc.vector.tensor_mul(yn[:], y_ps[:], rbc[:])
                ynorm.append(yn)
                fill(2)

            # ---- output projection: deferred as filler for the next
            # iteration's stall points ----
            fill_all()
            wo_state = {"steps": make_wo_steps(ynorm, qs0, last=(it == NQT - 1)),
                        "idx": 0}

        fill_all()

    nc.compile()
    return nc


def _host_inputs(x, w_q, w_kv_down, w_k_up, w_v_up, w_o):
    """Build the per-core input maps (host-side shard + precompute)."""
    import ml_dtypes
    BF = ml_dtypes.bfloat16
    E4 = (ml_dtypes.float8_e4m3fn if hasattr(ml_dtypes, "float8_e4m3fn")
          else ml_dtypes.float8_e4m3)
    WS = 64.0
    x = np.asarray(x)
    w_q = np.asarray(w_q)
    w_kv_down = np.asarray(w_kv_down)
    w_k_up = np.asarray(w_k_up)
    w_v_up = np.asarray(w_v_up)
    w_o = np.asarray(w_o)
    x2 = np.ascontiguousarray(x.reshape(T, C).astype(np.float32))
    xt = np.ascontiguousarray(x2.T)

    def hilo(a):
        hi = a.astype(E4)
        lo = (a - hi.astype(np.float32)).astype(E4)
        return np.ascontiguousarray(hi), np.ascontiguousarray(lo)

    xt_h, xt_l = hilo(xt)

    # RoPE tables, [hd, t] layout, sign folded into sin for the swapped term
    freqs = 1.0 / (ROPE_THETA ** (np.arange(0, HD, 2, dtype=np.float64) / HD))
    emb = np.arange(T, dtype=np.float64)[:, None] * freqs[None, :]   # [T, 64]
    cos = np.concatenate([np.cos(emb), np.cos(emb)], axis=-1)        # [T, 128]
    sin = np.concatenate([np.sin(emb), np.sin(emb)], axis=-1)
    cosT = np.ascontiguousarray(cos.T.astype(BF))                    # [128, T]
    sinS = sin.T.copy()
    sinS[:64, :] *= -1.0
    sinS = np.ascontiguousarray(sinS.astype(BF))

    # fixed triangular+global mask for the b-4 key block, [k_off, q_off]
    oi = np.arange(128)
    mTm = ((oi[None, :] <= oi[:, None]) | (oi[:, None] % 64 == 0)).astype(BF)

    # global-column mask [g, q]: visible iff 64 g < 128 (q//128 - 4)
    g = np.arange(NG)
    qb = np.arange(T) // BLOCK
    mGm = (64 * g[:, None] < 128 * (qb[None, :] - 4)).astype(BF)

    onesm = np.ones((128, 128), BF)
    ident = np.eye(128, dtype=BF)
    # swap matrix: out[m] = in[(m+64)%128]  (matmul form: swapm[k,m]=1 iff
    # k == (m+64)%128)
    km = np.arange(128)
    swapm = (km[:, None] == (km[None, :] + 64) % 128).astype(BF)

    wk_f = (w_kv_down.astype(np.float32) @ w_k_up.astype(np.float32))  # [C, KVH*HD]
    wv_f = (w_kv_down.astype(np.float32) @ w_v_up.astype(np.float32))

    in_maps = []
    for c in range(N_CORES):
        h0 = 2 * c
        kv = h0 // (H // KVH)
        wq0_h, wq0_l = hilo(w_q[:, h0 * HD:(h0 + 1) * HD].astype(np.float32) * WS)
        wq1_h, wq1_l = hilo(w_q[:, (h0 + 1) * HD:(h0 + 2) * HD].astype(np.float32) * WS)
        wk_h, wk_l = hilo(wk_f[:, kv * HD:(kv + 1) * HD] * WS)
        wv_h, wv_l = hilo(wv_f[:, kv * HD:(kv + 1) * HD] * WS)
        wo_c = np.ascontiguousarray(
            w_o[h0 * HD:(h0 + 2) * HD, :].astype(BF))
        in_maps.append({
            "xh": xt_h, "xl": xt_l,
            "wq0h": wq0_h, "wq0l": wq0_l, "wq1h": wq1_h, "wq1l": wq1_l,
            "wkh": wk_h, "wkl": wk_l, "wvh": wv_h, "wvl": wv_l,
            "wo": wo_c,
            "cosd": cosT, "sind": sinS, "maskt": mTm, "maskg": mGm,
            "onesd": onesm, "identd": ident, "swapd": swapm,
        })
    return in_maps


def _get_module():
    if "nc" not in _CACHE:
        _CACHE["nc"] = _build_module()
    return _CACHE["nc"]


def kernel(x, w_q, w_kv_down, w_k_up, w_v_up, w_o):
    from concourse.bass_utils import run_bass_kernel_spmd

    nc = _get_module()
    in_maps = _host_inputs(x, w_q, w_kv_down, w_k_up, w_v_up, w_o)
    res = run_bass_kernel_spmd(nc, in_maps, list(range(N_CORES)))
    acc = np.zeros((T, C), np.float32)
    for c in range(N_CORES):
        acc += np.asarray(res.results[c]["out"], dtype=np.float32)
    return acc.reshape(1, T, C)
